# revision 39
# baseline (speedup 1.0000x reference)
import sys

sys.path.insert(0, "/opt/trn_rl_repo")
import numpy as np
import ml_dtypes
import concourse.bass as bass
import concourse.mybir as mybir
import concourse.tile as tile
from concourse.bass_utils import run_bass_kernel_spmd

F32 = mybir.dt.float32
BF16 = mybir.dt.bfloat16
AF = mybir.ActivationFunctionType
ALU = mybir.AluOpType

C = 512
NH = 4          # heads per core (8 global, split in 2 groups of 4)
HD = 64
THETA = 10.0


import json as _json
import concourse.bass2jax as _b2j
import concourse.bass_utils as _bu

_ORIG_COMPILE = _bu.compile_bir_kernel


def _patched_compile_bir_kernel(bir_json, tmpdir, neff_name="file.neff"):
    """This walrus rejects instructions whose sync waits+updates exceed 2.
    Rewrite the BIR: move excess waits onto inserted same-engine Drains."""
    d = _json.loads(bir_json)
    for fn in d.get("functions", []):
        for b in fn.get("blocks", []):
            out = []
            for i in b.get("instructions", []):
                si = i.get("sync_info")
                if si:
                    ow = si.get("on_wait") or []
                    ou = si.get("on_update") or []
                    cap = 1 if i.get("opcode") in ("Drain", "Ldweights") else 2
                    budget = cap - len(ou)
                    if len(ow) > budget:
                        keep = ow[-budget:] if budget > 0 else []
                        extra = ow[:-budget] if budget > 0 else ow
                        for ci, w in enumerate(extra):
                            out.append({
                                "debug": i.get("debug", 0),
                                "engine": i["engine"],
                                "ins": [], "outs": [],
                                "name": f"{i['name']}sw{ci}",
                                "opcode": "Drain",
                                "sync_info": {"on_update": [],
                                              "on_wait": [w]},
                            })
                        si["on_wait"] = keep
                out.append(i)
            b["instructions"] = out
    return _ORIG_COMPILE(_json.dumps(d).encode(), tmpdir, neff_name=neff_name)


_bu.compile_bir_kernel = _patched_compile_bir_kernel
_b2j.compile_bir_kernel = _patched_compile_bir_kernel


def _build_nc(n_tok):
    nspan = n_tok // 512
    nc = bass.Bass()
    xT = nc.declare_dram_parameter("xT", [128, (n_tok // 512) * 2048], BF16, isOutput=False)
    w_q = nc.declare_dram_parameter("w_q", [4, 128, 256], BF16, isOutput=False)
    w_kv = nc.declare_dram_parameter("w_kv", [4, 128, 512], BF16, isOutput=False)
    bq = nc.declare_dram_parameter("bq", [128, 2], F32, isOutput=False)
    bkrow = nc.declare_dram_parameter("bkrow", [1, 512], BF16, isOutput=False)
    wp = nc.declare_dram_parameter("wp", [2, 128, 512], BF16, isOutput=False)
    # per-span rope tables (precomputed on host, DMA'd per span)
    tabs = nc.declare_dram_parameter("tabs", [128, nspan * 3072], BF16, isOutput=False)
    sel = nc.declare_dram_parameter("sel", [4, 2, 128], BF16, isOutput=False)
    mask0 = nc.declare_dram_parameter("mask0", [128, 128], BF16, isOutput=False)
    mask1 = nc.declare_dram_parameter("mask1", [128, 128], BF16, isOutput=False)
    dmask = nc.declare_dram_parameter("dmask", [128, 4], BF16, isOutput=False)
    y = nc.declare_dram_parameter("y", [n_tok, 512], BF16, isOutput=True)

    with nc.allow_low_precision(reason="bf16 pipeline by design"), tile.TileContext(nc) as tc:
        with tc.tile_pool(name="wpool", bufs=1) as wpool, \
             tc.tile_pool(name="store", bufs=1) as store:
            # ---- persistent tiles ----
            wq_t = wpool.tile([128, 4, 256], BF16, name="wq")
            wkv_t = wpool.tile([128, 4, 512], BF16, name="wkv")
            bq_t = wpool.tile([128, 2], F32, name="bq")
            bk_t = wpool.tile([1, 512], BF16, name="bk")
            wp_t = wpool.tile([128, 2, 512], BF16, name="wp")
            sel_t = wpool.tile([4, 2, 128], BF16, name="sel")
            mask0_t = wpool.tile([128, 128], BF16, name="mask0")
            mask1_t = wpool.tile([128, 128], BF16, name="mask1")
            dmask_t = wpool.tile([128, 4], BF16, name="dmask")
            ones_t = wpool.tile([1, 128], BF16, name="ones")

            nc.sync.dma_start(wq_t[:], w_q.rearrange("c p d -> p c d"))
            nc.sync.dma_start(wkv_t[:], w_kv.rearrange("c p d -> p c d"))
            nc.sync.dma_start(bq_t[:], bq[:])
            nc.sync.dma_start(bk_t[:], bkrow[:])
            nc.sync.dma_start(wp_t[:], wp.rearrange("e p c -> p e c"))
            nc.sync.dma_start(sel_t[:], sel[:])
            nc.sync.dma_start(mask0_t[:], mask0[:])
            nc.sync.dma_start(mask1_t[:], mask1[:])
            nc.sync.dma_start(dmask_t[:], dmask[:])
            nc.vector.memset(ones_t[:], 1.0)

            # q features for the whole sequence: [128, 2(R/I), n_tok] bf16
            q_store = store.tile([128, 2, n_tok], BF16, name="qs")

            # kv lhsT + den tiles (filled after pass 1)
            lR = [wpool.tile([128, 128], BF16, name=f"lR{i}") for i in range(2)]
            lI = [wpool.tile([128, 128], BF16, name=f"lI{i}") for i in range(2)]
            denR = wpool.tile([128, 4], BF16, name="denR")
            denI = wpool.tile([128, 4], BF16, name="denI")

            # ================ pass 1 ================
            with tc.tile_pool(name="kvacc", bufs=1, space="PSUM") as kvacc:
                kvR = kvacc.tile([128, 257], F32, name="kvR")
                kvI = kvacc.tile([128, 257], F32, name="kvI")
                with tc.tile_pool(name="p1", bufs=2) as p1, \
                     tc.tile_pool(name="kfp", bufs=3) as kfp, \
                     tc.tile_pool(name="xp", bufs=3) as xp, \
                     tc.tile_pool(name="qps", bufs=1, space="PSUM") as qps, \
                     tc.tile_pool(name="kvps", bufs=1, space="PSUM") as kvps:
                    kf_hist = [None, None, None]
                    va_hist = [None, None, None]
                    rot_hist = [None, None]
                    xt_tiles = {}
                    tab_tiles = {}

                    def dma_xt(s):
                        xt = xp.tile([128, 4, 512], BF16, name="xt")
                        nc.sync.dma_start(
                            xt[:], xT[:, s * 2048:(s + 1) * 2048].rearrange("p (c t) -> p c t", c=4))
                        xt_tiles[s] = xt

                    def dma_tab(s):
                        tab = p1.tile([128, 3072], BF16, name="tab")
                        nc.sync.dma_start(tab[:], tabs[:, s * 3072:(s + 1) * 3072])
                        tq1 = tab[:, 0:1024].rearrange("p (b t) -> p b t", b=2)
                        tq2 = tab[:, 1024:2048].rearrange("p (b t) -> p b t", b=2)
                        tkc = tab[:, 2048:2560].rearrange("p (t d) -> p t d", t=4)
                        tks = tab[:, 2560:3072].rearrange("p (t d) -> p t d", t=4)
                        tab_tiles[s] = (tq1, tq2, tkc, tks)

                    def stage_feat(s):
                        # elu(x)+1 = min(exp(x), max(x+1, 1))
                        rot_q, rot_k = rot_hist[s % 2]
                        exp_q = p1.tile([128, 2, 512], BF16, name="expq")
                        exp_k = p1.tile([128, 2, 512], BF16, name="expk")
                        nc.scalar.activation(exp_q[:], rot_q[:], AF.Exp)
                        nc.scalar.activation(exp_k[:], rot_k[:], AF.Exp)
                        a_q = p1.tile([128, 2, 512], BF16, name="aq")
                        a_k = p1.tile([128, 2, 512], BF16, name="ak")
                        nc.vector.tensor_scalar(a_q[:], rot_q[:], 1.0, 1.0,
                                                op0=ALU.add, op1=ALU.max)
                        nc.vector.tensor_scalar(a_k[:], rot_k[:], 1.0, 1.0,
                                                op0=ALU.add, op1=ALU.max)
                        nc.vector.tensor_tensor(
                            q_store[:, :, s * 512:(s + 1) * 512], exp_q[:], a_q[:], op=ALU.min)
                        # kf layout: [128 tok, 2 (R/I), 512 (t*128+d)]
                        kf = kfp.tile([128, 2, 512], BF16, name="kf")
                        nc.vector.tensor_tensor(kf[:], exp_k[:], a_k[:], op=ALU.min)
                        kf_hist[s % 3] = kf

                    def emit_kv(s):
                        bs = s % 3
                        kf = kf_hist[bs]
                        va = va_hist[bs]
                        for t in range(4):
                            st = (s == 0 and t == 0)
                            sp = (s == nspan - 1 and t == 3)
                            nc.tensor.matmul(kvR[:], kf[:, 0, t * 128:(t + 1) * 128],
                                             va[:, t, :], start=st, stop=sp)
                            nc.tensor.matmul(kvI[:], kf[:, 1, t * 128:(t + 1) * 128],
                                             va[:, t, :], start=st, stop=sp)

                    # prefetch: x tiles 2 spans ahead, tables 1 span ahead
                    dma_xt(0)
                    dma_xt(1)
                    dma_tab(0)
                    for s in range(nspan):
                        if s + 2 < nspan:
                            dma_xt(s + 2)
                        if s + 1 < nspan:
                            dma_tab(s + 1)
                        xt = xt_tiles.pop(s)
                        tq1, tq2, tkc, tks = tab_tiles.pop(s)

                        # q matmuls: out [128 d, 512 tok] per block, lhsT = w chunks
                        q_ps = qps.tile([128, 2, 512], F32, name="qp")
                        for blk in range(2):
                            for c in range(4):
                                nc.tensor.matmul(
                                    q_ps[:, blk, :], wq_t[:, c, blk * 128:(blk + 1) * 128],
                                    xt[:, c, :], start=(c == 0), stop=(c == 3))
                        # k+v matmuls: out [128 tok, 512 (kR kI v)] per t-tile
                        kv_ps = kvps.tile([128, 4, 512], F32, name="kvp")
                        for t in range(4):
                            for c in range(4):
                                nc.tensor.matmul(
                                    kv_ps[:, t, :], xt[:, c, t * 128:(t + 1) * 128],
                                    wkv_t[:, c, :], start=(c == 0), stop=False)
                            nc.tensor.matmul(kv_ps[:, t, :], ones_t[:], bk_t[:],
                                             start=False, stop=True)
                        # kv accumulation, two spans back (features guaranteed
                        # done; no PE stall)
                        if s > 1:
                            emit_kv(s - 2)

                        # psum -> sbuf copies (scalar), q gets bias folded in
                        q_sb = p1.tile([128, 2, 512], BF16, name="qsb")
                        for blk in range(2):
                            nc.scalar.activation(q_sb[:, blk, :], q_ps[:, blk, :],
                                                 AF.Identity, bias=bq_t[:, blk:blk + 1])
                        k_sb = p1.tile([128, 4, 256], BF16, name="ksb")
                        nc.scalar.copy(k_sb[:], kv_ps[:, :, 0:256])
                        va = kfp.tile([128, 4, 257], BF16, name="va")
                        nc.vector.memset(va[:, :, 256:257], 1.0)
                        nc.scalar.copy(va[:, :, 0:256], kv_ps[:, :, 256:512])

                        # ---- q rope (qT layout), paired ops ----
                        # tq1 = [c ; -s], tq2 = [s ; c]
                        P1 = p1.tile([128, 2, 512], BF16, name="P1")
                        P2 = p1.tile([128, 2, 512], BF16, name="P2")
                        nc.vector.tensor_tensor(P1[:], q_sb[:], tq1, op=ALU.mult)
                        nc.vector.tensor_tensor(P2[:], q_sb[:], tq2, op=ALU.mult)
                        rot_q = p1.tile([128, 2, 512], BF16, name="rotq")
                        nc.vector.tensor_tensor(rot_q[:, 0, :], P1[:, 0, :], P1[:, 1, :], op=ALU.add)
                        nc.vector.tensor_tensor(rot_q[:, 1, :], P2[:, 0, :], P2[:, 1, :], op=ALU.add)

                        # ---- k rope (token layout) ----
                        kR = k_sb[:, :, 0:128]
                        kI = k_sb[:, :, 128:256]
                        tk = p1.tile([128, 4, 512], BF16, name="tk")
                        tkv = [tk[:, i, :].rearrange("p (t d) -> p t d", d=128)
                               for i in range(4)]
                        nc.vector.tensor_tensor(tkv[0], kR, tkc, op=ALU.mult)
                        nc.vector.tensor_tensor(tkv[1], kI, tks, op=ALU.mult)
                        nc.vector.tensor_tensor(tkv[2], kR, tks, op=ALU.mult)
                        nc.vector.tensor_tensor(tkv[3], kI, tkc, op=ALU.mult)
                        rot_k = p1.tile([128, 2, 512], BF16, name="rotk")
                        nc.vector.tensor_tensor(rot_k[:, 0, :], tk[:, 0, :], tk[:, 1, :], op=ALU.subtract)
                        nc.vector.tensor_tensor(rot_k[:, 1, :], tk[:, 2, :], tk[:, 3, :], op=ALU.add)

                        rot_hist[s % 2] = (rot_q, rot_k)
                        va_hist[s % 3] = va
                        stage_feat(s)
                    emit_kv(nspan - 2)
                    emit_kv(nspan - 1)

                # ---- extract block-diag kv lhsT + den tiles via masks ----
                nc.vector.tensor_tensor(lR[0][:], kvR[:, 0:128], mask0_t[:], op=ALU.mult)
                nc.vector.tensor_tensor(lR[1][:], kvR[:, 128:256], mask1_t[:], op=ALU.mult)
                nc.vector.tensor_tensor(lI[0][:], kvI[:, 0:128], mask0_t[:], op=ALU.mult)
                nc.vector.tensor_tensor(lI[1][:], kvI[:, 128:256], mask1_t[:], op=ALU.mult)
                nc.vector.tensor_tensor(
                    denR[:], kvR[:, 256:257].broadcast_to([128, 4]), dmask_t[:], op=ALU.mult)
                nc.vector.tensor_tensor(
                    denI[:], kvI[:, 256:257].broadcast_to([128, 4]), dmask_t[:], op=ALU.mult)

            # ===== pass 2a: all denominators upfront (PE stays warm) =====
            zinv_all = store.tile([4, nspan, 512], BF16, name="zinva")
            with tc.tile_pool(name="p2a", bufs=3) as p2a, \
                 tc.tile_pool(name="dps", bufs=3, space="PSUM") as dps:
                for s in range(nspan):
                    sl = slice(s * 512, (s + 1) * 512)
                    den_ps = dps.tile([4, 512], F32, name="denp")
                    nc.tensor.matmul(den_ps[:], denR[:], q_store[:, 0, sl], start=True, stop=False)
                    nc.tensor.matmul(den_ps[:], denI[:], q_store[:, 1, sl], start=False, stop=True)
                    # 1/z = exp(-ln(z)); z is ~1e4 so edge cases are impossible
                    lnz = p2a.tile([4, 512], F32, name="lnz")
                    nc.scalar.activation(lnz[:], den_ps[:], AF.Ln)
                    nc.scalar.activation(zinv_all[:, s, :], lnz[:], AF.Exp, scale=-1.0)

            # ===== pass 2b: out, zb, proj (software pipelined) =====
            with tc.tile_pool(name="p2", bufs=3) as p2, \
                 tc.tile_pool(name="ops", bufs=2, space="PSUM") as ops, \
                 tc.tile_pool(name="zps", bufs=1, space="PSUM") as zps, \
                 tc.tile_pool(name="yps", bufs=2, space="PSUM") as yps:
                st = {}  # per-span live tiles

                def stage_a(s):
                    sl = slice(s * 512, (s + 1) * 512)
                    out_ps = ops.tile([128, 2, 512], F32, name="outp")
                    for i in range(2):
                        nc.tensor.matmul(out_ps[:, i, :], lR[i][:], q_store[:, 0, sl],
                                         start=True, stop=False)
                        nc.tensor.matmul(out_ps[:, i, :], lI[i][:], q_store[:, 1, sl],
                                         start=False, stop=True)
                    zb_ps = zps.tile([128, 2, 512], F32, name="zbp")
                    for i in range(2):
                        nc.tensor.matmul(zb_ps[:, i, :], sel_t[:, i, :], zinv_all[:, s, :],
                                         start=True, stop=True)
                    zb_sb = p2.tile([128, 2, 512], BF16, name="zbs")
                    nc.scalar.copy(zb_sb[:, 0, :], zb_ps[:, 0, :])
                    nc.vector.tensor_copy(zb_sb[:, 1, :], zb_ps[:, 1, :])
                    outT = p2.tile([128, 2, 512], BF16, name="outT")
                    for i in range(2):
                        nc.vector.tensor_tensor(outT[:, i, :], out_ps[:, i, :],
                                                zb_sb[:, i, :], op=ALU.mult)
                    st[s] = {"outT": outT}

                def stage_c(s):
                    d = st.pop(s)
                    outT = d["outT"]
                    y_sb = p2.tile([128, 4, 512], BF16, name="ysb")
                    for t in range(4):
                        y_ps = yps.tile([128, 512], F32, name="yp")
                        nc.tensor.matmul(y_ps[:], outT[:, 0, t * 128:(t + 1) * 128],
                                         wp_t[:, 0, :], start=True, stop=False)
                        nc.tensor.matmul(y_ps[:], outT[:, 1, t * 128:(t + 1) * 128],
                                         wp_t[:, 1, :], start=False, stop=True)
                        nc.scalar.copy(y_sb[:, t, :], y_ps[:])
                    nc.sync.dma_start(
                        y[s * 512:(s + 1) * 512, :].rearrange("(t p) c -> p t c", p=128),
                        y_sb[:])

                for s in range(nspan):
                    stage_a(s)
                    if s >= 2:
                        stage_c(s - 2)
                stage_c(nspan - 2)
                stage_c(nspan - 1)

    return nc


_NC_CACHE = {}


def _get_nc(n_tok):
    if n_tok not in _NC_CACHE:
        _NC_CACHE[n_tok] = _build_nc(n_tok)
    return _NC_CACHE[n_tok]


def _bf(a):
    return np.ascontiguousarray(np.asarray(a, dtype=np.float32)).astype(ml_dtypes.bfloat16)


_TABLES_CACHE = {}


def _tables(nspan):
    """Per-span rope tables, shared by all cores.

    qt1/qt2 [128 d, nspan, 2 blk, 512 tok']: q rotation in qT layout,
      qt1 = [cos ; -sin], qt2 = [sin ; cos].
    ktc/kts [128 tok, nspan, 4 t, 128 d]: k rotation in token layout.
    """
    if nspan in _TABLES_CACHE:
        return _TABLES_CACHE[nspan]
    j = np.arange(16)
    freqs = (1.0 / (THETA ** (4.0 * j / HD))).astype(np.float64)
    fcol = np.tile(freqs, 4)                      # [64] head-major 4h x 16f
    tx = np.arange(128).astype(np.float64)
    angx = np.outer(fcol, tx)                     # [64 f, 128 tx]
    rows = np.arange(nspan * 4).astype(np.float64)
    angy = np.outer(fcol, rows)                   # [64 f, nspan*4]

    # q tables [128, nspan, 4, 128]
    qc = np.empty((128, nspan, 4, 128), np.float32)
    qs_ = np.empty((128, nspan, 4, 128), np.float32)
    qc[0:64] = np.cos(angx)[:, None, None, :]
    qs_[0:64] = np.sin(angx)[:, None, None, :]
    qc[64:128] = np.cos(angy).reshape(64, nspan, 4, 1)
    qs_[64:128] = np.sin(angy).reshape(64, nspan, 4, 1)
    # stack blocks: [128, nspan, 2, 4*128]
    qcf = qc.reshape(128, nspan, 1, 512)
    qsf = qs_.reshape(128, nspan, 1, 512)
    qt1 = np.concatenate([qcf, -qsf], axis=2).reshape(128, nspan * 1024)
    qt2 = np.concatenate([qsf, qcf], axis=2).reshape(128, nspan * 1024)

    # k tables [128 tok(part), nspan, 4 t, 128 d]
    kc = np.empty((128, nspan, 4, 128), np.float32)
    ks = np.empty((128, nspan, 4, 128), np.float32)
    kc[:, :, :, 0:64] = np.cos(angx).T[:, None, None, :]
    ks[:, :, :, 0:64] = np.sin(angx).T[:, None, None, :]
    kc[:, :, :, 64:128] = np.cos(angy).T.reshape(1, nspan, 4, 64)
    ks[:, :, :, 64:128] = np.sin(angy).T.reshape(1, nspan, 4, 64)
    ktc = kc.reshape(128, nspan, 512)
    kts = ks.reshape(128, nspan, 512)
    tabs = np.concatenate([qt1.reshape(128, nspan, 1024),
                           qt2.reshape(128, nspan, 1024), ktc, kts],
                          axis=2).reshape(128, nspan * 3072)
    out = _bf(tabs)
    _TABLES_CACHE[nspan] = out
    return out


def kernel(x, w_qkv, b_qkv, w_proj, b_proj, height, width):
    x = np.asarray(x); w_qkv = np.asarray(w_qkv); b_qkv = np.asarray(b_qkv)
    w_proj = np.asarray(w_proj); b_proj = np.asarray(b_proj)
    b, n, c = x.shape
    nc = _get_nc(n)
    tabs = _tables(n // 512)

    sel = np.zeros((4, 2, 128), np.float32)
    for i in range(2):
        for h in range(2):
            sel[2 * i + h, i, 64 * h:64 * h + 64] = 1.0
    # head of partition p in the d layout: (p % 64) // 16
    hop = (np.arange(128) % 64) // 16
    mask0 = (hop[:, None] == (np.arange(128) // 64)[None, :]).astype(np.float32)
    mask1 = (hop[:, None] == (2 + np.arange(128) // 64)[None, :]).astype(np.float32)
    dmask = (hop[:, None] == np.arange(4)[None, :]).astype(np.float32)

    in_maps = []
    for core in range(8):
        bi, hg = core // 2, core % 2
        heads = [hg * NH + j for j in range(NH)]
        q0 = [h * HD + 2 * j for h in heads for j in range(16)] + \
             [h * HD + 32 + 2 * j for h in heads for j in range(16)]
        q1 = [cc + 1 for cc in q0]
        kR = [512 + cc for cc in q0]
        kI = [512 + cc for cc in q1]
        vc = [1024 + h * HD + e for h in heads for e in range(HD)]
        wq_cols = q0 + q1
        wkv_cols = kR + kI + vc
        bk = np.concatenate([b_qkv[kR + kI], np.zeros(256, np.float32)])
        in_maps.append({
            "xT": _bf(x[bi].T.reshape(4, 128, n // 512, 512).transpose(1, 2, 0, 3).reshape(128, -1)),
            "w_q": _bf(w_qkv[:, wq_cols]).reshape(4, 128, 256),
            "w_kv": _bf(w_qkv[:, wkv_cols]).reshape(4, 128, 512),
            "bq": np.stack([b_qkv[q0], b_qkv[q1]], axis=1).astype(np.float32),
            "bkrow": _bf(bk)[None, :],
            "wp": _bf(np.stack([w_proj[hg * 256:hg * 256 + 128, :],
                                w_proj[hg * 256 + 128:hg * 256 + 256, :]])),
            "tabs": tabs,
            "sel": _bf(sel),
            "mask0": _bf(mask0), "mask1": _bf(mask1), "dmask": _bf(dmask),
        })
    res = run_bass_kernel_spmd(nc, in_maps, list(range(8)), trace=False)
    bias_eff = (b_proj.astype(np.float32)
                + b_qkv[1024:].astype(np.float32) @ w_proj.astype(np.float32))
    out = np.empty((b, n, c), np.float32)
    for bi in range(b):
        out[bi] = (res.results[2 * bi]["y"].astype(np.float32)
                   + res.results[2 * bi + 1]["y"].astype(np.float32)
                   + bias_eff[None, :])
    return out


# revision 40
# speedup vs baseline: 1.1894x; 1.1894x over previous
import sys

sys.path.insert(0, "/opt/trn_rl_repo")
import numpy as np
import ml_dtypes
import concourse.bass as bass
import concourse.mybir as mybir
import concourse.tile as tile
from concourse.bass_utils import run_bass_kernel_spmd

F32 = mybir.dt.float32
BF16 = mybir.dt.bfloat16
AF = mybir.ActivationFunctionType
ALU = mybir.AluOpType

C = 512
NH = 4          # heads per core (8 global, split in 2 groups of 4)
HD = 64
THETA = 10.0


import json as _json
import concourse.bass2jax as _b2j
import concourse.bass_utils as _bu

_ORIG_COMPILE = _bu.compile_bir_kernel


def _patched_compile_bir_kernel(bir_json, tmpdir, neff_name="file.neff"):
    """This walrus rejects instructions whose sync waits+updates exceed 2.
    Rewrite the BIR: move excess waits onto inserted same-engine Drains."""
    d = _json.loads(bir_json)
    for fn in d.get("functions", []):
        for b in fn.get("blocks", []):
            out = []
            for i in b.get("instructions", []):
                si = i.get("sync_info")
                if si:
                    ow = si.get("on_wait") or []
                    ou = si.get("on_update") or []
                    cap = 1 if i.get("opcode") in ("Drain", "Ldweights") else 2
                    budget = cap - len(ou)
                    if len(ow) > budget:
                        keep = ow[-budget:] if budget > 0 else []
                        extra = ow[:-budget] if budget > 0 else ow
                        for ci, w in enumerate(extra):
                            out.append({
                                "debug": i.get("debug", 0),
                                "engine": i["engine"],
                                "ins": [], "outs": [],
                                "name": f"{i['name']}sw{ci}",
                                "opcode": "Drain",
                                "sync_info": {"on_update": [],
                                              "on_wait": [w]},
                            })
                        si["on_wait"] = keep
                out.append(i)
            b["instructions"] = out
    return _ORIG_COMPILE(_json.dumps(d).encode(), tmpdir, neff_name=neff_name)


_bu.compile_bir_kernel = _patched_compile_bir_kernel
_b2j.compile_bir_kernel = _patched_compile_bir_kernel


def _build_nc(n_tok):
    nspan = n_tok // 512
    nc = bass.Bass()
    xT = nc.declare_dram_parameter("xT", [128, (n_tok // 512) * 2048], BF16, isOutput=False)
    w_q = nc.declare_dram_parameter("w_q", [4, 128, 256], BF16, isOutput=False)
    w_kv = nc.declare_dram_parameter("w_kv", [4, 128, 512], BF16, isOutput=False)
    bq = nc.declare_dram_parameter("bq", [128, 2], F32, isOutput=False)
    bkrow = nc.declare_dram_parameter("bkrow", [1, 512], BF16, isOutput=False)
    wp = nc.declare_dram_parameter("wp", [2, 128, 512], BF16, isOutput=False)
    # per-span rope tables (precomputed on host, DMA'd per span)
    qt1 = nc.declare_dram_parameter("qt1", [128, nspan * 1024], BF16, isOutput=False)
    qt2 = nc.declare_dram_parameter("qt2", [128, nspan * 1024], BF16, isOutput=False)
    ktc = nc.declare_dram_parameter("ktc", [128, nspan * 512], BF16, isOutput=False)
    kts = nc.declare_dram_parameter("kts", [128, nspan * 512], BF16, isOutput=False)
    sel = nc.declare_dram_parameter("sel", [4, 2, 128], BF16, isOutput=False)
    mask0 = nc.declare_dram_parameter("mask0", [128, 128], BF16, isOutput=False)
    mask1 = nc.declare_dram_parameter("mask1", [128, 128], BF16, isOutput=False)
    dmask = nc.declare_dram_parameter("dmask", [128, 4], BF16, isOutput=False)
    y = nc.declare_dram_parameter("y", [n_tok, 512], BF16, isOutput=True)

    with nc.allow_low_precision(reason="bf16 pipeline by design"), tile.TileContext(nc) as tc:
        with tc.tile_pool(name="wpool", bufs=1) as wpool, \
             tc.tile_pool(name="store", bufs=1) as store:
            # ---- persistent tiles ----
            wq_t = wpool.tile([128, 4, 256], BF16, name="wq")
            wkv_t = wpool.tile([128, 4, 512], BF16, name="wkv")
            bq_t = wpool.tile([128, 2], F32, name="bq")
            bk_t = wpool.tile([1, 512], BF16, name="bk")
            wp_t = wpool.tile([128, 2, 512], BF16, name="wp")
            sel_t = wpool.tile([4, 2, 128], BF16, name="sel")
            mask0_t = wpool.tile([128, 128], BF16, name="mask0")
            mask1_t = wpool.tile([128, 128], BF16, name="mask1")
            dmask_t = wpool.tile([128, 4], BF16, name="dmask")
            ones_t = wpool.tile([1, 128], BF16, name="ones")

            nc.sync.dma_start(wq_t[:], w_q.rearrange("c p d -> p c d"))
            nc.sync.dma_start(wkv_t[:], w_kv.rearrange("c p d -> p c d"))
            nc.sync.dma_start(bq_t[:], bq[:])
            nc.sync.dma_start(bk_t[:], bkrow[:])
            nc.sync.dma_start(wp_t[:], wp.rearrange("e p c -> p e c"))
            nc.sync.dma_start(sel_t[:], sel[:])
            nc.sync.dma_start(mask0_t[:], mask0[:])
            nc.sync.dma_start(mask1_t[:], mask1[:])
            nc.sync.dma_start(dmask_t[:], dmask[:])
            nc.vector.memset(ones_t[:], 1.0)

            # q features for the whole sequence: [128, 2(R/I), n_tok] bf16
            q_store = store.tile([128, 2, n_tok], BF16, name="qs")

            # kv lhsT + den tiles (filled after pass 1)
            lR = [wpool.tile([128, 128], BF16, name=f"lR{i}") for i in range(2)]
            lI = [wpool.tile([128, 128], BF16, name=f"lI{i}") for i in range(2)]
            denR = wpool.tile([128, 4], BF16, name="denR")
            denI = wpool.tile([128, 4], BF16, name="denI")

            # ================ pass 1 ================
            with tc.tile_pool(name="kvacc", bufs=1, space="PSUM") as kvacc:
                kvR = kvacc.tile([128, 257], F32, name="kvR")
                kvI = kvacc.tile([128, 257], F32, name="kvI")
                with tc.tile_pool(name="p1", bufs=2) as p1, \
                     tc.tile_pool(name="kfp", bufs=3) as kfp, \
                     tc.tile_pool(name="xp", bufs=3) as xp, \
                     tc.tile_pool(name="qps", bufs=1, space="PSUM") as qps, \
                     tc.tile_pool(name="kvps", bufs=1, space="PSUM") as kvps:
                    kf_hist = [None, None, None]
                    va_hist = [None, None, None]
                    rot_hist = [None, None]
                    xt_tiles = {}
                    tab_tiles = {}

                    def dma_xt(s):
                        xt = xp.tile([128, 4, 512], BF16, name="xt")
                        nc.sync.dma_start(
                            xt[:], xT[:, s * 2048:(s + 1) * 2048].rearrange("p (c t) -> p c t", c=4))
                        xt_tiles[s] = xt

                    def dma_tab(s):
                        tq1 = p1.tile([128, 2, 512], BF16, name="tq1")
                        tq2 = p1.tile([128, 2, 512], BF16, name="tq2")
                        tkc = p1.tile([128, 4, 128], BF16, name="tkc")
                        tks = p1.tile([128, 4, 128], BF16, name="tks")
                        nc.sync.dma_start(
                            tq1[:], qt1[:, s * 1024:(s + 1) * 1024].rearrange("p (b t) -> p b t", b=2))
                        nc.sync.dma_start(
                            tq2[:], qt2[:, s * 1024:(s + 1) * 1024].rearrange("p (b t) -> p b t", b=2))
                        nc.sync.dma_start(
                            tkc[:], ktc[:, s * 512:(s + 1) * 512].rearrange("p (t d) -> p t d", t=4))
                        nc.sync.dma_start(
                            tks[:], kts[:, s * 512:(s + 1) * 512].rearrange("p (t d) -> p t d", t=4))
                        tab_tiles[s] = (tq1, tq2, tkc, tks)

                    def stage_feat(s):
                        # elu(x)+1 = min(exp(x), max(x+1, 1))
                        rot_q, rot_k = rot_hist[s % 2]
                        exp_q = p1.tile([128, 2, 512], BF16, name="expq")
                        exp_k = p1.tile([128, 2, 512], BF16, name="expk")
                        nc.scalar.activation(exp_q[:], rot_q[:], AF.Exp)
                        nc.scalar.activation(exp_k[:], rot_k[:], AF.Exp)
                        a_q = p1.tile([128, 2, 512], BF16, name="aq")
                        a_k = p1.tile([128, 2, 512], BF16, name="ak")
                        nc.vector.tensor_scalar(a_q[:], rot_q[:], 1.0, 1.0,
                                                op0=ALU.add, op1=ALU.max)
                        nc.vector.tensor_scalar(a_k[:], rot_k[:], 1.0, 1.0,
                                                op0=ALU.add, op1=ALU.max)
                        nc.vector.tensor_tensor(
                            q_store[:, :, s * 512:(s + 1) * 512], exp_q[:], a_q[:], op=ALU.min)
                        # kf layout: [128 tok, 2 (R/I), 512 (t*128+d)]
                        kf = kfp.tile([128, 2, 512], BF16, name="kf")
                        nc.vector.tensor_tensor(kf[:], exp_k[:], a_k[:], op=ALU.min)
                        kf_hist[s % 3] = kf

                    def emit_kv(s):
                        bs = s % 3
                        kf = kf_hist[bs]
                        va = va_hist[bs]
                        for t in range(4):
                            st = (s == 0 and t == 0)
                            sp = (s == nspan - 1 and t == 3)
                            nc.tensor.matmul(kvR[:], kf[:, 0, t * 128:(t + 1) * 128],
                                             va[:, t, :], start=st, stop=sp)
                            nc.tensor.matmul(kvI[:], kf[:, 1, t * 128:(t + 1) * 128],
                                             va[:, t, :], start=st, stop=sp)

                    # prefetch: x tiles 2 spans ahead, tables 1 span ahead
                    dma_xt(0)
                    dma_xt(1)
                    dma_tab(0)
                    for s in range(nspan):
                        if s + 2 < nspan:
                            dma_xt(s + 2)
                        if s + 1 < nspan:
                            dma_tab(s + 1)
                        xt = xt_tiles.pop(s)
                        tq1, tq2, tkc, tks = tab_tiles.pop(s)

                        # q matmuls: out [128 d, 512 tok] per block, lhsT = w chunks
                        q_ps = qps.tile([128, 2, 512], F32, name="qp")
                        for blk in range(2):
                            for c in range(4):
                                nc.tensor.matmul(
                                    q_ps[:, blk, :], wq_t[:, c, blk * 128:(blk + 1) * 128],
                                    xt[:, c, :], start=(c == 0), stop=(c == 3))
                        # k+v matmuls: out [128 tok, 512 (kR kI v)] per t-tile
                        kv_ps = kvps.tile([128, 4, 512], F32, name="kvp")
                        for t in range(4):
                            for c in range(4):
                                nc.tensor.matmul(
                                    kv_ps[:, t, :], xt[:, c, t * 128:(t + 1) * 128],
                                    wkv_t[:, c, :], start=(c == 0), stop=False)
                            nc.tensor.matmul(kv_ps[:, t, :], ones_t[:], bk_t[:],
                                             start=False, stop=True)
                        # kv accumulation, two spans back (features guaranteed
                        # done; no PE stall)
                        if s > 1:
                            emit_kv(s - 2)

                        # psum -> sbuf copies (scalar), q gets bias folded in
                        q_sb = p1.tile([128, 2, 512], BF16, name="qsb")
                        for blk in range(2):
                            nc.scalar.activation(q_sb[:, blk, :], q_ps[:, blk, :],
                                                 AF.Identity, bias=bq_t[:, blk:blk + 1])
                        k_sb = p1.tile([128, 4, 256], BF16, name="ksb")
                        nc.scalar.copy(k_sb[:], kv_ps[:, :, 0:256])
                        va = kfp.tile([128, 4, 257], BF16, name="va")
                        nc.vector.memset(va[:, :, 256:257], 1.0)
                        nc.scalar.copy(va[:, :, 0:256], kv_ps[:, :, 256:512])

                        # ---- q rope (qT layout), paired ops ----
                        # tq1 = [c ; -s], tq2 = [s ; c]
                        P1 = p1.tile([128, 2, 512], BF16, name="P1")
                        P2 = p1.tile([128, 2, 512], BF16, name="P2")
                        nc.vector.tensor_tensor(P1[:], q_sb[:], tq1[:], op=ALU.mult)
                        nc.vector.tensor_tensor(P2[:], q_sb[:], tq2[:], op=ALU.mult)
                        rot_q = p1.tile([128, 2, 512], BF16, name="rotq")
                        nc.vector.tensor_tensor(rot_q[:, 0, :], P1[:, 0, :], P1[:, 1, :], op=ALU.add)
                        nc.vector.tensor_tensor(rot_q[:, 1, :], P2[:, 0, :], P2[:, 1, :], op=ALU.add)

                        # ---- k rope (token layout) ----
                        kR = k_sb[:, :, 0:128]
                        kI = k_sb[:, :, 128:256]
                        tk = p1.tile([128, 4, 512], BF16, name="tk")
                        tkv = [tk[:, i, :].rearrange("p (t d) -> p t d", d=128)
                               for i in range(4)]
                        nc.vector.tensor_tensor(tkv[0], kR, tkc[:], op=ALU.mult)
                        nc.vector.tensor_tensor(tkv[1], kI, tks[:], op=ALU.mult)
                        nc.vector.tensor_tensor(tkv[2], kR, tks[:], op=ALU.mult)
                        nc.vector.tensor_tensor(tkv[3], kI, tkc[:], op=ALU.mult)
                        rot_k = p1.tile([128, 2, 512], BF16, name="rotk")
                        nc.vector.tensor_tensor(rot_k[:, 0, :], tk[:, 0, :], tk[:, 1, :], op=ALU.subtract)
                        nc.vector.tensor_tensor(rot_k[:, 1, :], tk[:, 2, :], tk[:, 3, :], op=ALU.add)

                        rot_hist[s % 2] = (rot_q, rot_k)
                        va_hist[s % 3] = va
                        stage_feat(s)
                    emit_kv(nspan - 2)
                    emit_kv(nspan - 1)

                # ---- extract block-diag kv lhsT + den tiles via masks ----
                nc.vector.tensor_tensor(lR[0][:], kvR[:, 0:128], mask0_t[:], op=ALU.mult)
                nc.vector.tensor_tensor(lR[1][:], kvR[:, 128:256], mask1_t[:], op=ALU.mult)
                nc.vector.tensor_tensor(lI[0][:], kvI[:, 0:128], mask0_t[:], op=ALU.mult)
                nc.vector.tensor_tensor(lI[1][:], kvI[:, 128:256], mask1_t[:], op=ALU.mult)
                nc.vector.tensor_tensor(
                    denR[:], kvR[:, 256:257].broadcast_to([128, 4]), dmask_t[:], op=ALU.mult)
                nc.vector.tensor_tensor(
                    denI[:], kvI[:, 256:257].broadcast_to([128, 4]), dmask_t[:], op=ALU.mult)

            # ===== pass 2a: all denominators upfront (PE stays warm) =====
            zinv_all = store.tile([4, nspan, 512], BF16, name="zinva")
            with tc.tile_pool(name="p2a", bufs=3) as p2a, \
                 tc.tile_pool(name="dps", bufs=3, space="PSUM") as dps:
                for s in range(nspan):
                    sl = slice(s * 512, (s + 1) * 512)
                    den_ps = dps.tile([4, 512], F32, name="denp")
                    nc.tensor.matmul(den_ps[:], denR[:], q_store[:, 0, sl], start=True, stop=False)
                    nc.tensor.matmul(den_ps[:], denI[:], q_store[:, 1, sl], start=False, stop=True)
                    # 1/z = exp(-ln(z)); z is ~1e4 so edge cases are impossible
                    lnz = p2a.tile([4, 512], F32, name="lnz")
                    nc.scalar.activation(lnz[:], den_ps[:], AF.Ln)
                    nc.scalar.activation(zinv_all[:, s, :], lnz[:], AF.Exp, scale=-1.0)

            # ===== pass 2b: out, zb, proj (software pipelined) =====
            with tc.tile_pool(name="p2", bufs=3) as p2, \
                 tc.tile_pool(name="ops", bufs=2, space="PSUM") as ops, \
                 tc.tile_pool(name="zps", bufs=1, space="PSUM") as zps, \
                 tc.tile_pool(name="yps", bufs=2, space="PSUM") as yps:
                st = {}  # per-span live tiles

                def stage_a(s):
                    sl = slice(s * 512, (s + 1) * 512)
                    out_ps = ops.tile([128, 2, 512], F32, name="outp")
                    for i in range(2):
                        nc.tensor.matmul(out_ps[:, i, :], lR[i][:], q_store[:, 0, sl],
                                         start=True, stop=False)
                        nc.tensor.matmul(out_ps[:, i, :], lI[i][:], q_store[:, 1, sl],
                                         start=False, stop=True)
                    zb_ps = zps.tile([128, 2, 512], F32, name="zbp")
                    for i in range(2):
                        nc.tensor.matmul(zb_ps[:, i, :], sel_t[:, i, :], zinv_all[:, s, :],
                                         start=True, stop=True)
                    zb_sb = p2.tile([128, 2, 512], BF16, name="zbs")
                    nc.scalar.copy(zb_sb[:, 0, :], zb_ps[:, 0, :])
                    nc.vector.tensor_copy(zb_sb[:, 1, :], zb_ps[:, 1, :])
                    outT = p2.tile([128, 2, 512], BF16, name="outT")
                    for i in range(2):
                        nc.vector.tensor_tensor(outT[:, i, :], out_ps[:, i, :],
                                                zb_sb[:, i, :], op=ALU.mult)
                    st[s] = {"outT": outT}

                def stage_c(s):
                    d = st.pop(s)
                    outT = d["outT"]
                    y_sb = p2.tile([128, 4, 512], BF16, name="ysb")
                    for t in range(4):
                        y_ps = yps.tile([128, 512], F32, name="yp")
                        nc.tensor.matmul(y_ps[:], outT[:, 0, t * 128:(t + 1) * 128],
                                         wp_t[:, 0, :], start=True, stop=False)
                        nc.tensor.matmul(y_ps[:], outT[:, 1, t * 128:(t + 1) * 128],
                                         wp_t[:, 1, :], start=False, stop=True)
                        nc.scalar.copy(y_sb[:, t, :], y_ps[:])
                    nc.sync.dma_start(
                        y[s * 512:(s + 1) * 512, :].rearrange("(t p) c -> p t c", p=128),
                        y_sb[:])

                for s in range(nspan):
                    stage_a(s)
                    if s >= 2:
                        stage_c(s - 2)
                stage_c(nspan - 2)
                stage_c(nspan - 1)

    return nc


_NC_CACHE = {}


def _get_nc(n_tok):
    if n_tok not in _NC_CACHE:
        _NC_CACHE[n_tok] = _build_nc(n_tok)
    return _NC_CACHE[n_tok]


def _bf(a):
    return np.ascontiguousarray(np.asarray(a, dtype=np.float32)).astype(ml_dtypes.bfloat16)


_TABLES_CACHE = {}


def _tables(nspan):
    """Per-span rope tables, shared by all cores.

    qt1/qt2 [128 d, nspan, 2 blk, 512 tok']: q rotation in qT layout,
      qt1 = [cos ; -sin], qt2 = [sin ; cos].
    ktc/kts [128 tok, nspan, 4 t, 128 d]: k rotation in token layout.
    """
    if nspan in _TABLES_CACHE:
        return _TABLES_CACHE[nspan]
    j = np.arange(16)
    freqs = (1.0 / (THETA ** (4.0 * j / HD))).astype(np.float64)
    fcol = np.tile(freqs, 4)                      # [64] head-major 4h x 16f
    tx = np.arange(128).astype(np.float64)
    angx = np.outer(fcol, tx)                     # [64 f, 128 tx]
    rows = np.arange(nspan * 4).astype(np.float64)
    angy = np.outer(fcol, rows)                   # [64 f, nspan*4]

    # q tables [128, nspan, 4, 128]
    qc = np.empty((128, nspan, 4, 128), np.float32)
    qs_ = np.empty((128, nspan, 4, 128), np.float32)
    qc[0:64] = np.cos(angx)[:, None, None, :]
    qs_[0:64] = np.sin(angx)[:, None, None, :]
    qc[64:128] = np.cos(angy).reshape(64, nspan, 4, 1)
    qs_[64:128] = np.sin(angy).reshape(64, nspan, 4, 1)
    # stack blocks: [128, nspan, 2, 4*128]
    qcf = qc.reshape(128, nspan, 1, 512)
    qsf = qs_.reshape(128, nspan, 1, 512)
    qt1 = np.concatenate([qcf, -qsf], axis=2).reshape(128, nspan * 1024)
    qt2 = np.concatenate([qsf, qcf], axis=2).reshape(128, nspan * 1024)

    # k tables [128 tok(part), nspan, 4 t, 128 d]
    kc = np.empty((128, nspan, 4, 128), np.float32)
    ks = np.empty((128, nspan, 4, 128), np.float32)
    kc[:, :, :, 0:64] = np.cos(angx).T[:, None, None, :]
    ks[:, :, :, 0:64] = np.sin(angx).T[:, None, None, :]
    kc[:, :, :, 64:128] = np.cos(angy).T.reshape(1, nspan, 4, 64)
    ks[:, :, :, 64:128] = np.sin(angy).T.reshape(1, nspan, 4, 64)
    ktc = kc.reshape(128, nspan * 512)
    kts = ks.reshape(128, nspan * 512)
    out = (_bf(qt1), _bf(qt2), _bf(ktc), _bf(kts))
    _TABLES_CACHE[nspan] = out
    return out


def kernel(x, w_qkv, b_qkv, w_proj, b_proj, height, width):
    x = np.asarray(x); w_qkv = np.asarray(w_qkv); b_qkv = np.asarray(b_qkv)
    w_proj = np.asarray(w_proj); b_proj = np.asarray(b_proj)
    b, n, c = x.shape
    nc = _get_nc(n)
    qt1, qt2, ktc, kts = _tables(n // 512)

    sel = np.zeros((4, 2, 128), np.float32)
    for i in range(2):
        for h in range(2):
            sel[2 * i + h, i, 64 * h:64 * h + 64] = 1.0
    # head of partition p in the d layout: (p % 64) // 16
    hop = (np.arange(128) % 64) // 16
    mask0 = (hop[:, None] == (np.arange(128) // 64)[None, :]).astype(np.float32)
    mask1 = (hop[:, None] == (2 + np.arange(128) // 64)[None, :]).astype(np.float32)
    dmask = (hop[:, None] == np.arange(4)[None, :]).astype(np.float32)

    in_maps = []
    for core in range(8):
        bi, hg = core // 2, core % 2
        heads = [hg * NH + j for j in range(NH)]
        q0 = [h * HD + 2 * j for h in heads for j in range(16)] + \
             [h * HD + 32 + 2 * j for h in heads for j in range(16)]
        q1 = [cc + 1 for cc in q0]
        kR = [512 + cc for cc in q0]
        kI = [512 + cc for cc in q1]
        vc = [1024 + h * HD + e for h in heads for e in range(HD)]
        wq_cols = q0 + q1
        wkv_cols = kR + kI + vc
        bk = np.concatenate([b_qkv[kR + kI], np.zeros(256, np.float32)])
        in_maps.append({
            "xT": _bf(x[bi].T.reshape(4, 128, n // 512, 512).transpose(1, 2, 0, 3).reshape(128, -1)),
            "w_q": _bf(w_qkv[:, wq_cols]).reshape(4, 128, 256),
            "w_kv": _bf(w_qkv[:, wkv_cols]).reshape(4, 128, 512),
            "bq": np.stack([b_qkv[q0], b_qkv[q1]], axis=1).astype(np.float32),
            "bkrow": _bf(bk)[None, :],
            "wp": _bf(np.stack([w_proj[hg * 256:hg * 256 + 128, :],
                                w_proj[hg * 256 + 128:hg * 256 + 256, :]])),
            "qt1": qt1, "qt2": qt2, "ktc": ktc, "kts": kts,
            "sel": _bf(sel),
            "mask0": _bf(mask0), "mask1": _bf(mask1), "dmask": _bf(dmask),
        })
    res = run_bass_kernel_spmd(nc, in_maps, list(range(8)), trace=False)
    bias_eff = (b_proj.astype(np.float32)
                + b_qkv[1024:].astype(np.float32) @ w_proj.astype(np.float32))
    out = np.empty((b, n, c), np.float32)
    for bi in range(b):
        out[bi] = (res.results[2 * bi]["y"].astype(np.float32)
                   + res.results[2 * bi + 1]["y"].astype(np.float32)
                   + bias_eff[None, :])
    return out


# revision 41
# speedup vs baseline: 1.1933x; 1.0033x over previous
import sys

sys.path.insert(0, "/opt/trn_rl_repo")
import numpy as np
import ml_dtypes
import concourse.bass as bass
import concourse.mybir as mybir
import concourse.tile as tile
from concourse.bass_utils import run_bass_kernel_spmd

F32 = mybir.dt.float32
BF16 = mybir.dt.bfloat16
AF = mybir.ActivationFunctionType
ALU = mybir.AluOpType

C = 512
NH = 4          # heads per core (8 global, split in 2 groups of 4)
HD = 64
THETA = 10.0


import json as _json
import concourse.bass2jax as _b2j
import concourse.bass_utils as _bu

_ORIG_COMPILE = _bu.compile_bir_kernel


def _patched_compile_bir_kernel(bir_json, tmpdir, neff_name="file.neff"):
    """This walrus rejects instructions whose sync waits+updates exceed 2.
    Rewrite the BIR: move excess waits onto inserted same-engine Drains."""
    d = _json.loads(bir_json)
    for fn in d.get("functions", []):
        for b in fn.get("blocks", []):
            out = []
            for i in b.get("instructions", []):
                si = i.get("sync_info")
                if si:
                    ow = si.get("on_wait") or []
                    ou = si.get("on_update") or []
                    cap = 1 if i.get("opcode") in ("Drain", "Ldweights") else 2
                    budget = cap - len(ou)
                    if len(ow) > budget:
                        keep = ow[-budget:] if budget > 0 else []
                        extra = ow[:-budget] if budget > 0 else ow
                        for ci, w in enumerate(extra):
                            out.append({
                                "debug": i.get("debug", 0),
                                "engine": i["engine"],
                                "ins": [], "outs": [],
                                "name": f"{i['name']}sw{ci}",
                                "opcode": "Drain",
                                "sync_info": {"on_update": [],
                                              "on_wait": [w]},
                            })
                        si["on_wait"] = keep
                out.append(i)
            b["instructions"] = out
    return _ORIG_COMPILE(_json.dumps(d).encode(), tmpdir, neff_name=neff_name)


_bu.compile_bir_kernel = _patched_compile_bir_kernel
_b2j.compile_bir_kernel = _patched_compile_bir_kernel


def _build_nc(n_tok):
    nspan = n_tok // 512
    nc = bass.Bass()
    xT = nc.declare_dram_parameter("xT", [128, (n_tok // 512) * 2048], BF16, isOutput=False)
    w_q = nc.declare_dram_parameter("w_q", [4, 128, 256], BF16, isOutput=False)
    w_kv = nc.declare_dram_parameter("w_kv", [4, 128, 512], BF16, isOutput=False)
    bq = nc.declare_dram_parameter("bq", [128, 2], F32, isOutput=False)
    bkrow = nc.declare_dram_parameter("bkrow", [1, 512], BF16, isOutput=False)
    wp = nc.declare_dram_parameter("wp", [2, 128, 512], BF16, isOutput=False)
    # per-span rope tables (precomputed on host, DMA'd per span)
    qt1 = nc.declare_dram_parameter("qt1", [128, nspan * 1024], BF16, isOutput=False)
    qt2 = nc.declare_dram_parameter("qt2", [128, nspan * 1024], BF16, isOutput=False)
    ktc = nc.declare_dram_parameter("ktc", [128, nspan * 512], BF16, isOutput=False)
    kts = nc.declare_dram_parameter("kts", [128, nspan * 512], BF16, isOutput=False)
    sel = nc.declare_dram_parameter("sel", [4, 2, 128], BF16, isOutput=False)
    mask0 = nc.declare_dram_parameter("mask0", [128, 128], BF16, isOutput=False)
    mask1 = nc.declare_dram_parameter("mask1", [128, 128], BF16, isOutput=False)
    dmask = nc.declare_dram_parameter("dmask", [128, 4], BF16, isOutput=False)
    y = nc.declare_dram_parameter("y", [n_tok, 512], BF16, isOutput=True)

    with nc.allow_low_precision(reason="bf16 pipeline by design"), tile.TileContext(nc) as tc:
        with tc.tile_pool(name="wpool", bufs=1) as wpool, \
             tc.tile_pool(name="store", bufs=1) as store:
            # ---- persistent tiles ----
            wq_t = wpool.tile([128, 4, 256], BF16, name="wq")
            wkv_t = wpool.tile([128, 4, 512], BF16, name="wkv")
            bq_t = wpool.tile([128, 2], F32, name="bq")
            bk_t = wpool.tile([1, 512], BF16, name="bk")
            wp_t = wpool.tile([128, 2, 512], BF16, name="wp")
            sel_t = wpool.tile([4, 2, 128], BF16, name="sel")
            mask0_t = wpool.tile([128, 128], BF16, name="mask0")
            mask1_t = wpool.tile([128, 128], BF16, name="mask1")
            dmask_t = wpool.tile([128, 4], BF16, name="dmask")
            ones_t = wpool.tile([1, 128], BF16, name="ones")

            nc.sync.dma_start(wq_t[:], w_q.rearrange("c p d -> p c d"))
            nc.sync.dma_start(wkv_t[:], w_kv.rearrange("c p d -> p c d"))
            nc.sync.dma_start(bq_t[:], bq[:])
            nc.sync.dma_start(bk_t[:], bkrow[:])
            nc.sync.dma_start(wp_t[:], wp.rearrange("e p c -> p e c"))
            nc.sync.dma_start(sel_t[:], sel[:])
            nc.sync.dma_start(mask0_t[:], mask0[:])
            nc.sync.dma_start(mask1_t[:], mask1[:])
            nc.sync.dma_start(dmask_t[:], dmask[:])
            nc.vector.memset(ones_t[:], 1.0)

            # q features for the whole sequence: [128, 2(R/I), n_tok] bf16
            q_store = store.tile([128, 2, n_tok], BF16, name="qs")

            # kv lhsT + den tiles (filled after pass 1)
            lR = [wpool.tile([128, 128], BF16, name=f"lR{i}") for i in range(2)]
            lI = [wpool.tile([128, 128], BF16, name=f"lI{i}") for i in range(2)]
            denR = wpool.tile([128, 4], BF16, name="denR")
            denI = wpool.tile([128, 4], BF16, name="denI")

            # ================ pass 1 ================
            with tc.tile_pool(name="kvacc", bufs=1, space="PSUM") as kvacc:
                kvR = kvacc.tile([128, 257], F32, name="kvR")
                kvI = kvacc.tile([128, 257], F32, name="kvI")
                with tc.tile_pool(name="p1", bufs=2) as p1, \
                     tc.tile_pool(name="kfp", bufs=3) as kfp, \
                     tc.tile_pool(name="xp", bufs=3) as xp, \
                     tc.tile_pool(name="qps", bufs=1, space="PSUM") as qps, \
                     tc.tile_pool(name="kvps", bufs=1, space="PSUM") as kvps:
                    kf_hist = [None, None, None]
                    va_hist = [None, None, None]
                    rot_hist = [None, None]
                    xt_tiles = {}
                    tab_tiles = {}

                    def dma_xt(s):
                        xt = xp.tile([128, 4, 512], BF16, name="xt")
                        nc.sync.dma_start(
                            xt[:], xT[:, s * 2048:(s + 1) * 2048].rearrange("p (c t) -> p c t", c=4))
                        xt_tiles[s] = xt

                    def dma_tab(s):
                        tq1 = p1.tile([128, 2, 512], BF16, name="tq1")
                        tq2 = p1.tile([128, 2, 512], BF16, name="tq2")
                        tkc = p1.tile([128, 4, 128], BF16, name="tkc")
                        tks = p1.tile([128, 4, 128], BF16, name="tks")
                        nc.sync.dma_start(
                            tq1[:], qt1[:, s * 1024:(s + 1) * 1024].rearrange("p (b t) -> p b t", b=2))
                        nc.sync.dma_start(
                            tq2[:], qt2[:, s * 1024:(s + 1) * 1024].rearrange("p (b t) -> p b t", b=2))
                        nc.sync.dma_start(
                            tkc[:], ktc[:, s * 512:(s + 1) * 512].rearrange("p (t d) -> p t d", t=4))
                        nc.sync.dma_start(
                            tks[:], kts[:, s * 512:(s + 1) * 512].rearrange("p (t d) -> p t d", t=4))
                        tab_tiles[s] = (tq1, tq2, tkc, tks)

                    def stage_feat(s):
                        # elu(x)+1 = min(exp(x), max(x+1, 1))
                        rot_q, rot_k = rot_hist[s % 2]
                        exp_q = p1.tile([128, 2, 512], BF16, name="expq")
                        exp_k = p1.tile([128, 2, 512], BF16, name="expk")
                        nc.scalar.activation(exp_q[:], rot_q[:], AF.Exp)
                        nc.scalar.activation(exp_k[:], rot_k[:], AF.Exp)
                        a_q = p1.tile([128, 2, 512], BF16, name="aq")
                        a_k = p1.tile([128, 2, 512], BF16, name="ak")
                        nc.vector.tensor_scalar(a_q[:], rot_q[:], 1.0, 1.0,
                                                op0=ALU.add, op1=ALU.max)
                        nc.vector.tensor_scalar(a_k[:], rot_k[:], 1.0, 1.0,
                                                op0=ALU.add, op1=ALU.max)
                        nc.vector.tensor_tensor(
                            q_store[:, :, s * 512:(s + 1) * 512], exp_q[:], a_q[:], op=ALU.min)
                        # kf layout: [128 tok, 2 (R/I), 512 (t*128+d)]
                        kf = kfp.tile([128, 2, 512], BF16, name="kf")
                        nc.vector.tensor_tensor(kf[:], exp_k[:], a_k[:], op=ALU.min)
                        kf_hist[s % 3] = kf

                    def emit_kv(s):
                        bs = s % 3
                        kf = kf_hist[bs]
                        va = va_hist[bs]
                        for t in range(4):
                            st = (s == 0 and t == 0)
                            sp = (s == nspan - 1 and t == 3)
                            nc.tensor.matmul(kvR[:], kf[:, 0, t * 128:(t + 1) * 128],
                                             va[:, t, :], start=st, stop=sp)
                            nc.tensor.matmul(kvI[:], kf[:, 1, t * 128:(t + 1) * 128],
                                             va[:, t, :], start=st, stop=sp)

                    # prefetch: x tiles 2 spans ahead, tables 1 span ahead
                    dma_xt(0)
                    dma_xt(1)
                    dma_tab(0)
                    for s in range(nspan):
                        if s + 2 < nspan:
                            dma_xt(s + 2)
                        if s + 1 < nspan:
                            dma_tab(s + 1)
                        xt = xt_tiles.pop(s)
                        tq1, tq2, tkc, tks = tab_tiles.pop(s)

                        # q matmuls: out [128 d, 512 tok] per block, lhsT = w chunks
                        q_ps = qps.tile([128, 2, 512], F32, name="qp")
                        for blk in range(2):
                            for c in range(4):
                                nc.tensor.matmul(
                                    q_ps[:, blk, :], wq_t[:, c, blk * 128:(blk + 1) * 128],
                                    xt[:, c, :], start=(c == 0), stop=(c == 3))
                        # k+v matmuls: out [128 tok, 512 (kR kI v)] per t-tile
                        kv_ps = kvps.tile([128, 4, 512], F32, name="kvp")
                        for t in range(4):
                            for c in range(4):
                                nc.tensor.matmul(
                                    kv_ps[:, t, :], xt[:, c, t * 128:(t + 1) * 128],
                                    wkv_t[:, c, :], start=(c == 0), stop=False)
                            nc.tensor.matmul(kv_ps[:, t, :], ones_t[:], bk_t[:],
                                             start=False, stop=True)
                        # kv accumulation, two spans back (features guaranteed
                        # done; no PE stall)
                        if s > 1:
                            emit_kv(s - 2)

                        # psum -> sbuf copies (scalar), q gets bias folded in
                        q_sb = p1.tile([128, 2, 512], BF16, name="qsb")
                        for blk in range(2):
                            nc.scalar.activation(q_sb[:, blk, :], q_ps[:, blk, :],
                                                 AF.Identity, bias=bq_t[:, blk:blk + 1])
                        k_sb = p1.tile([128, 4, 256], BF16, name="ksb")
                        nc.scalar.copy(k_sb[:], kv_ps[:, :, 0:256])
                        va = kfp.tile([128, 4, 257], BF16, name="va")
                        nc.vector.memset(va[:, :, 256:257], 1.0)
                        nc.scalar.copy(va[:, :, 0:256], kv_ps[:, :, 256:512])

                        # ---- q rope (qT layout), paired ops ----
                        # tq1 = [c ; -s], tq2 = [s ; c]
                        P1 = p1.tile([128, 2, 512], BF16, name="P1")
                        P2 = p1.tile([128, 2, 512], BF16, name="P2")
                        nc.vector.tensor_tensor(P1[:], q_sb[:], tq1[:], op=ALU.mult)
                        nc.vector.tensor_tensor(P2[:], q_sb[:], tq2[:], op=ALU.mult)
                        rot_q = p1.tile([128, 2, 512], BF16, name="rotq")
                        nc.vector.tensor_tensor(rot_q[:, 0, :], P1[:, 0, :], P1[:, 1, :], op=ALU.add)
                        nc.vector.tensor_tensor(rot_q[:, 1, :], P2[:, 0, :], P2[:, 1, :], op=ALU.add)

                        # ---- k rope (token layout) ----
                        kR = k_sb[:, :, 0:128]
                        kI = k_sb[:, :, 128:256]
                        tk = p1.tile([128, 4, 512], BF16, name="tk")
                        tkv = [tk[:, i, :].rearrange("p (t d) -> p t d", d=128)
                               for i in range(4)]
                        nc.vector.tensor_tensor(tkv[0], kR, tkc[:], op=ALU.mult)
                        nc.vector.tensor_tensor(tkv[1], kI, tks[:], op=ALU.mult)
                        nc.vector.tensor_tensor(tkv[2], kR, tks[:], op=ALU.mult)
                        nc.vector.tensor_tensor(tkv[3], kI, tkc[:], op=ALU.mult)
                        rot_k = p1.tile([128, 2, 512], BF16, name="rotk")
                        nc.vector.tensor_tensor(rot_k[:, 0, :], tk[:, 0, :], tk[:, 1, :], op=ALU.subtract)
                        nc.vector.tensor_tensor(rot_k[:, 1, :], tk[:, 2, :], tk[:, 3, :], op=ALU.add)

                        rot_hist[s % 2] = (rot_q, rot_k)
                        va_hist[s % 3] = va
                        stage_feat(s)
                    emit_kv(nspan - 2)
                    emit_kv(nspan - 1)

                # ---- extract block-diag kv lhsT + den tiles via masks ----
                nc.vector.tensor_tensor(lR[0][:], kvR[:, 0:128], mask0_t[:], op=ALU.mult)
                nc.vector.tensor_tensor(lR[1][:], kvR[:, 128:256], mask1_t[:], op=ALU.mult)
                nc.vector.tensor_tensor(lI[0][:], kvI[:, 0:128], mask0_t[:], op=ALU.mult)
                nc.vector.tensor_tensor(lI[1][:], kvI[:, 128:256], mask1_t[:], op=ALU.mult)
                nc.vector.tensor_tensor(
                    denR[:], kvR[:, 256:257].broadcast_to([128, 4]), dmask_t[:], op=ALU.mult)
                nc.vector.tensor_tensor(
                    denI[:], kvI[:, 256:257].broadcast_to([128, 4]), dmask_t[:], op=ALU.mult)

            # ===== pass 2a: all denominators upfront (PE stays warm) =====
            # chunked tiles so 2b can start before all of 2a has finished
            zchunk = 8
            zinv_tiles = [store.tile([4, zchunk, 512], BF16, name=f"zinva{i}")
                          for i in range(nspan // zchunk)]
            with tc.tile_pool(name="p2a", bufs=3) as p2a, \
                 tc.tile_pool(name="dps", bufs=3, space="PSUM") as dps:
                for s in range(nspan):
                    sl = slice(s * 512, (s + 1) * 512)
                    den_ps = dps.tile([4, 512], F32, name="denp")
                    nc.tensor.matmul(den_ps[:], denR[:], q_store[:, 0, sl], start=True, stop=False)
                    nc.tensor.matmul(den_ps[:], denI[:], q_store[:, 1, sl], start=False, stop=True)
                    # 1/z = exp(-ln(z)); z is ~1e4 so edge cases are impossible
                    lnz = p2a.tile([4, 512], F32, name="lnz")
                    nc.scalar.activation(lnz[:], den_ps[:], AF.Ln)
                    nc.scalar.activation(zinv_tiles[s // zchunk][:, s % zchunk, :],
                                         lnz[:], AF.Exp, scale=-1.0)

            # ===== pass 2b: out, zb, proj (software pipelined) =====
            with tc.tile_pool(name="p2", bufs=3) as p2, \
                 tc.tile_pool(name="ops", bufs=2, space="PSUM") as ops, \
                 tc.tile_pool(name="zps", bufs=1, space="PSUM") as zps, \
                 tc.tile_pool(name="yps", bufs=2, space="PSUM") as yps:
                st = {}  # per-span live tiles

                def stage_a(s):
                    sl = slice(s * 512, (s + 1) * 512)
                    out_ps = ops.tile([128, 2, 512], F32, name="outp")
                    for i in range(2):
                        nc.tensor.matmul(out_ps[:, i, :], lR[i][:], q_store[:, 0, sl],
                                         start=True, stop=False)
                        nc.tensor.matmul(out_ps[:, i, :], lI[i][:], q_store[:, 1, sl],
                                         start=False, stop=True)
                    zb_ps = zps.tile([128, 2, 512], F32, name="zbp")
                    for i in range(2):
                        nc.tensor.matmul(zb_ps[:, i, :], sel_t[:, i, :],
                                         zinv_tiles[s // zchunk][:, s % zchunk, :],
                                         start=True, stop=True)
                    zb_sb = p2.tile([128, 2, 512], BF16, name="zbs")
                    nc.scalar.copy(zb_sb[:, 0, :], zb_ps[:, 0, :])
                    nc.vector.tensor_copy(zb_sb[:, 1, :], zb_ps[:, 1, :])
                    outT = p2.tile([128, 2, 512], BF16, name="outT")
                    for i in range(2):
                        nc.vector.tensor_tensor(outT[:, i, :], out_ps[:, i, :],
                                                zb_sb[:, i, :], op=ALU.mult)
                    st[s] = {"outT": outT}

                def stage_c(s):
                    d = st.pop(s)
                    outT = d["outT"]
                    y_sb = p2.tile([128, 4, 512], BF16, name="ysb")
                    for t in range(4):
                        y_ps = yps.tile([128, 512], F32, name="yp")
                        nc.tensor.matmul(y_ps[:], outT[:, 0, t * 128:(t + 1) * 128],
                                         wp_t[:, 0, :], start=True, stop=False)
                        nc.tensor.matmul(y_ps[:], outT[:, 1, t * 128:(t + 1) * 128],
                                         wp_t[:, 1, :], start=False, stop=True)
                        nc.scalar.copy(y_sb[:, t, :], y_ps[:])
                    nc.sync.dma_start(
                        y[s * 512:(s + 1) * 512, :].rearrange("(t p) c -> p t c", p=128),
                        y_sb[:])

                for s in range(nspan):
                    stage_a(s)
                    if s >= 2:
                        stage_c(s - 2)
                stage_c(nspan - 2)
                stage_c(nspan - 1)

    return nc


_NC_CACHE = {}


def _get_nc(n_tok):
    if n_tok not in _NC_CACHE:
        _NC_CACHE[n_tok] = _build_nc(n_tok)
    return _NC_CACHE[n_tok]


def _bf(a):
    return np.ascontiguousarray(np.asarray(a, dtype=np.float32)).astype(ml_dtypes.bfloat16)


_TABLES_CACHE = {}


def _tables(nspan):
    """Per-span rope tables, shared by all cores.

    qt1/qt2 [128 d, nspan, 2 blk, 512 tok']: q rotation in qT layout,
      qt1 = [cos ; -sin], qt2 = [sin ; cos].
    ktc/kts [128 tok, nspan, 4 t, 128 d]: k rotation in token layout.
    """
    if nspan in _TABLES_CACHE:
        return _TABLES_CACHE[nspan]
    j = np.arange(16)
    freqs = (1.0 / (THETA ** (4.0 * j / HD))).astype(np.float64)
    fcol = np.tile(freqs, 4)                      # [64] head-major 4h x 16f
    tx = np.arange(128).astype(np.float64)
    angx = np.outer(fcol, tx)                     # [64 f, 128 tx]
    rows = np.arange(nspan * 4).astype(np.float64)
    angy = np.outer(fcol, rows)                   # [64 f, nspan*4]

    # q tables [128, nspan, 4, 128]
    qc = np.empty((128, nspan, 4, 128), np.float32)
    qs_ = np.empty((128, nspan, 4, 128), np.float32)
    qc[0:64] = np.cos(angx)[:, None, None, :]
    qs_[0:64] = np.sin(angx)[:, None, None, :]
    qc[64:128] = np.cos(angy).reshape(64, nspan, 4, 1)
    qs_[64:128] = np.sin(angy).reshape(64, nspan, 4, 1)
    # stack blocks: [128, nspan, 2, 4*128]
    qcf = qc.reshape(128, nspan, 1, 512)
    qsf = qs_.reshape(128, nspan, 1, 512)
    qt1 = np.concatenate([qcf, -qsf], axis=2).reshape(128, nspan * 1024)
    qt2 = np.concatenate([qsf, qcf], axis=2).reshape(128, nspan * 1024)

    # k tables [128 tok(part), nspan, 4 t, 128 d]
    kc = np.empty((128, nspan, 4, 128), np.float32)
    ks = np.empty((128, nspan, 4, 128), np.float32)
    kc[:, :, :, 0:64] = np.cos(angx).T[:, None, None, :]
    ks[:, :, :, 0:64] = np.sin(angx).T[:, None, None, :]
    kc[:, :, :, 64:128] = np.cos(angy).T.reshape(1, nspan, 4, 64)
    ks[:, :, :, 64:128] = np.sin(angy).T.reshape(1, nspan, 4, 64)
    ktc = kc.reshape(128, nspan * 512)
    kts = ks.reshape(128, nspan * 512)
    out = (_bf(qt1), _bf(qt2), _bf(ktc), _bf(kts))
    _TABLES_CACHE[nspan] = out
    return out


def kernel(x, w_qkv, b_qkv, w_proj, b_proj, height, width):
    x = np.asarray(x); w_qkv = np.asarray(w_qkv); b_qkv = np.asarray(b_qkv)
    w_proj = np.asarray(w_proj); b_proj = np.asarray(b_proj)
    b, n, c = x.shape
    nc = _get_nc(n)
    qt1, qt2, ktc, kts = _tables(n // 512)

    sel = np.zeros((4, 2, 128), np.float32)
    for i in range(2):
        for h in range(2):
            sel[2 * i + h, i, 64 * h:64 * h + 64] = 1.0
    # head of partition p in the d layout: (p % 64) // 16
    hop = (np.arange(128) % 64) // 16
    mask0 = (hop[:, None] == (np.arange(128) // 64)[None, :]).astype(np.float32)
    mask1 = (hop[:, None] == (2 + np.arange(128) // 64)[None, :]).astype(np.float32)
    dmask = (hop[:, None] == np.arange(4)[None, :]).astype(np.float32)

    in_maps = []
    for core in range(8):
        bi, hg = core // 2, core % 2
        heads = [hg * NH + j for j in range(NH)]
        q0 = [h * HD + 2 * j for h in heads for j in range(16)] + \
             [h * HD + 32 + 2 * j for h in heads for j in range(16)]
        q1 = [cc + 1 for cc in q0]
        kR = [512 + cc for cc in q0]
        kI = [512 + cc for cc in q1]
        vc = [1024 + h * HD + e for h in heads for e in range(HD)]
        wq_cols = q0 + q1
        wkv_cols = kR + kI + vc
        bk = np.concatenate([b_qkv[kR + kI], np.zeros(256, np.float32)])
        in_maps.append({
            "xT": _bf(x[bi].T.reshape(4, 128, n // 512, 512).transpose(1, 2, 0, 3).reshape(128, -1)),
            "w_q": _bf(w_qkv[:, wq_cols]).reshape(4, 128, 256),
            "w_kv": _bf(w_qkv[:, wkv_cols]).reshape(4, 128, 512),
            "bq": np.stack([b_qkv[q0], b_qkv[q1]], axis=1).astype(np.float32),
            "bkrow": _bf(bk)[None, :],
            "wp": _bf(np.stack([w_proj[hg * 256:hg * 256 + 128, :],
                                w_proj[hg * 256 + 128:hg * 256 + 256, :]])),
            "qt1": qt1, "qt2": qt2, "ktc": ktc, "kts": kts,
            "sel": _bf(sel),
            "mask0": _bf(mask0), "mask1": _bf(mask1), "dmask": _bf(dmask),
        })
    res = run_bass_kernel_spmd(nc, in_maps, list(range(8)), trace=False)
    bias_eff = (b_proj.astype(np.float32)
                + b_qkv[1024:].astype(np.float32) @ w_proj.astype(np.float32))
    out = np.empty((b, n, c), np.float32)
    for bi in range(b):
        out[bi] = (res.results[2 * bi]["y"].astype(np.float32)
                   + res.results[2 * bi + 1]["y"].astype(np.float32)
                   + bias_eff[None, :])
    return out


# revision 42
# speedup vs baseline: 1.1943x; 1.0008x over previous
import sys

sys.path.insert(0, "/opt/trn_rl_repo")
import numpy as np
import ml_dtypes
import concourse.bass as bass
import concourse.mybir as mybir
import concourse.tile as tile
from concourse.bass_utils import run_bass_kernel_spmd

F32 = mybir.dt.float32
BF16 = mybir.dt.bfloat16
AF = mybir.ActivationFunctionType
ALU = mybir.AluOpType

C = 512
NH = 4          # heads per core (8 global, split in 2 groups of 4)
HD = 64
THETA = 10.0


import json as _json
import concourse.bass2jax as _b2j
import concourse.bass_utils as _bu

_ORIG_COMPILE = _bu.compile_bir_kernel


def _patched_compile_bir_kernel(bir_json, tmpdir, neff_name="file.neff"):
    """This walrus rejects instructions whose sync waits+updates exceed 2.
    Rewrite the BIR: move excess waits onto inserted same-engine Drains."""
    d = _json.loads(bir_json)
    for fn in d.get("functions", []):
        for b in fn.get("blocks", []):
            out = []
            for i in b.get("instructions", []):
                si = i.get("sync_info")
                if si:
                    ow = si.get("on_wait") or []
                    ou = si.get("on_update") or []
                    cap = 1 if i.get("opcode") in ("Drain", "Ldweights") else 2
                    budget = cap - len(ou)
                    if len(ow) > budget:
                        keep = ow[-budget:] if budget > 0 else []
                        extra = ow[:-budget] if budget > 0 else ow
                        for ci, w in enumerate(extra):
                            out.append({
                                "debug": i.get("debug", 0),
                                "engine": i["engine"],
                                "ins": [], "outs": [],
                                "name": f"{i['name']}sw{ci}",
                                "opcode": "Drain",
                                "sync_info": {"on_update": [],
                                              "on_wait": [w]},
                            })
                        si["on_wait"] = keep
                out.append(i)
            b["instructions"] = out
    return _ORIG_COMPILE(_json.dumps(d).encode(), tmpdir, neff_name=neff_name)


_bu.compile_bir_kernel = _patched_compile_bir_kernel
_b2j.compile_bir_kernel = _patched_compile_bir_kernel


def _build_nc(n_tok):
    nspan = n_tok // 512
    nc = bass.Bass()
    xT = nc.declare_dram_parameter("xT", [128, (n_tok // 512) * 2048], BF16, isOutput=False)
    w_q = nc.declare_dram_parameter("w_q", [4, 128, 256], BF16, isOutput=False)
    w_kv = nc.declare_dram_parameter("w_kv", [4, 128, 512], BF16, isOutput=False)
    bq = nc.declare_dram_parameter("bq", [128, 2], F32, isOutput=False)
    bkrow = nc.declare_dram_parameter("bkrow", [1, 512], BF16, isOutput=False)
    wp = nc.declare_dram_parameter("wp", [2, 128, 512], BF16, isOutput=False)
    # per-span rope tables (precomputed on host, DMA'd per span)
    qt1 = nc.declare_dram_parameter("qt1", [128, nspan * 1024], BF16, isOutput=False)
    qt2 = nc.declare_dram_parameter("qt2", [128, nspan * 1024], BF16, isOutput=False)
    ktc = nc.declare_dram_parameter("ktc", [128, nspan * 512], BF16, isOutput=False)
    kts = nc.declare_dram_parameter("kts", [128, nspan * 512], BF16, isOutput=False)
    sel = nc.declare_dram_parameter("sel", [4, 2, 128], BF16, isOutput=False)
    mask0 = nc.declare_dram_parameter("mask0", [128, 128], BF16, isOutput=False)
    mask1 = nc.declare_dram_parameter("mask1", [128, 128], BF16, isOutput=False)
    dmask = nc.declare_dram_parameter("dmask", [128, 4], BF16, isOutput=False)
    y = nc.declare_dram_parameter("y", [n_tok, 512], BF16, isOutput=True)

    with nc.allow_low_precision(reason="bf16 pipeline by design"), tile.TileContext(nc) as tc:
        with tc.tile_pool(name="wpool", bufs=1) as wpool, \
             tc.tile_pool(name="store", bufs=1) as store:
            # ---- persistent tiles ----
            wq_t = wpool.tile([128, 4, 256], BF16, name="wq")
            wkv_t = wpool.tile([128, 4, 512], BF16, name="wkv")
            bq_t = wpool.tile([128, 2], F32, name="bq")
            bk_t = wpool.tile([1, 512], BF16, name="bk")
            wp_t = wpool.tile([128, 2, 512], BF16, name="wp")
            sel_t = wpool.tile([4, 2, 128], BF16, name="sel")
            mask0_t = wpool.tile([128, 128], BF16, name="mask0")
            mask1_t = wpool.tile([128, 128], BF16, name="mask1")
            dmask_t = wpool.tile([128, 4], BF16, name="dmask")
            ones_t = wpool.tile([1, 128], BF16, name="ones")

            nc.sync.dma_start(wq_t[:], w_q.rearrange("c p d -> p c d"))
            nc.sync.dma_start(wkv_t[:], w_kv.rearrange("c p d -> p c d"))
            nc.sync.dma_start(bq_t[:], bq[:])
            nc.sync.dma_start(bk_t[:], bkrow[:])
            nc.sync.dma_start(wp_t[:], wp.rearrange("e p c -> p e c"))
            nc.sync.dma_start(sel_t[:], sel[:])
            nc.sync.dma_start(mask0_t[:], mask0[:])
            nc.sync.dma_start(mask1_t[:], mask1[:])
            nc.sync.dma_start(dmask_t[:], dmask[:])
            nc.vector.memset(ones_t[:], 1.0)

            # q features for the whole sequence: [128, 2(R/I), n_tok] bf16
            q_store = store.tile([128, 2, n_tok], BF16, name="qs")

            # kv lhsT + den tiles (filled after pass 1)
            lR = [wpool.tile([128, 128], BF16, name=f"lR{i}") for i in range(2)]
            lI = [wpool.tile([128, 128], BF16, name=f"lI{i}") for i in range(2)]
            denR = wpool.tile([128, 4], BF16, name="denR")
            denI = wpool.tile([128, 4], BF16, name="denI")

            # ================ pass 1 ================
            with tc.tile_pool(name="kvacc", bufs=1, space="PSUM") as kvacc:
                kvR = kvacc.tile([128, 257], F32, name="kvR")
                kvI = kvacc.tile([128, 257], F32, name="kvI")
                with tc.tile_pool(name="p1", bufs=2) as p1, \
                     tc.tile_pool(name="kfp", bufs=3) as kfp, \
                     tc.tile_pool(name="xp", bufs=3) as xp, \
                     tc.tile_pool(name="qps", bufs=1, space="PSUM") as qps, \
                     tc.tile_pool(name="kvps", bufs=1, space="PSUM") as kvps:
                    kf_hist = [None, None, None]
                    va_hist = [None, None, None]
                    rot_hist = [None, None]
                    xt_tiles = {}
                    tab_tiles = {}

                    def dma_xt(s):
                        xt = xp.tile([128, 4, 512], BF16, name="xt")
                        nc.sync.dma_start(
                            xt[:], xT[:, s * 2048:(s + 1) * 2048].rearrange("p (c t) -> p c t", c=4))
                        xt_tiles[s] = xt

                    def dma_tab(s):
                        tq1 = p1.tile([128, 2, 512], BF16, name="tq1")
                        tq2 = p1.tile([128, 2, 512], BF16, name="tq2")
                        tkc = p1.tile([128, 4, 128], BF16, name="tkc")
                        tks = p1.tile([128, 4, 128], BF16, name="tks")
                        nc.sync.dma_start(
                            tq1[:], qt1[:, s * 1024:(s + 1) * 1024].rearrange("p (b t) -> p b t", b=2))
                        nc.sync.dma_start(
                            tq2[:], qt2[:, s * 1024:(s + 1) * 1024].rearrange("p (b t) -> p b t", b=2))
                        nc.sync.dma_start(
                            tkc[:], ktc[:, s * 512:(s + 1) * 512].rearrange("p (t d) -> p t d", t=4))
                        nc.sync.dma_start(
                            tks[:], kts[:, s * 512:(s + 1) * 512].rearrange("p (t d) -> p t d", t=4))
                        tab_tiles[s] = (tq1, tq2, tkc, tks)

                    def stage_feat(s):
                        # elu(x)+1 = min(exp(x), max(x+1, 1))
                        rot_q, rot_k = rot_hist[s % 2]
                        exp_q = p1.tile([128, 2, 512], BF16, name="expq")
                        exp_k = p1.tile([128, 2, 512], BF16, name="expk")
                        nc.scalar.activation(exp_q[:], rot_q[:], AF.Exp)
                        nc.scalar.activation(exp_k[:], rot_k[:], AF.Exp)
                        a_q = p1.tile([128, 2, 512], BF16, name="aq")
                        a_k = p1.tile([128, 2, 512], BF16, name="ak")
                        nc.vector.tensor_scalar(a_q[:], rot_q[:], 1.0, 1.0,
                                                op0=ALU.add, op1=ALU.max)
                        nc.vector.tensor_scalar(a_k[:], rot_k[:], 1.0, 1.0,
                                                op0=ALU.add, op1=ALU.max)
                        nc.vector.tensor_tensor(
                            q_store[:, :, s * 512:(s + 1) * 512], exp_q[:], a_q[:], op=ALU.min)
                        # kf layout: [128 tok, 2 (R/I), 512 (t*128+d)]
                        kf = kfp.tile([128, 2, 512], BF16, name="kf")
                        nc.vector.tensor_tensor(kf[:], exp_k[:], a_k[:], op=ALU.min)
                        kf_hist[s % 3] = kf

                    def emit_kv(s):
                        bs = s % 3
                        kf = kf_hist[bs]
                        va = va_hist[bs]
                        for t in range(4):
                            st = (s == 0 and t == 0)
                            sp = (s == nspan - 1 and t == 3)
                            nc.tensor.matmul(kvR[:], kf[:, 0, t * 128:(t + 1) * 128],
                                             va[:, t, :], start=st, stop=sp)
                            nc.tensor.matmul(kvI[:], kf[:, 1, t * 128:(t + 1) * 128],
                                             va[:, t, :], start=st, stop=sp)

                    # prefetch: x tiles 2 spans ahead, tables 1 span ahead
                    dma_xt(0)
                    dma_xt(1)
                    dma_tab(0)
                    for s in range(nspan):
                        if s + 2 < nspan:
                            dma_xt(s + 2)
                        if s + 1 < nspan:
                            dma_tab(s + 1)
                        xt = xt_tiles.pop(s)
                        tq1, tq2, tkc, tks = tab_tiles.pop(s)

                        # q matmuls: out [128 d, 512 tok] per block, lhsT = w chunks
                        q_ps = qps.tile([128, 2, 512], F32, name="qp")
                        for blk in range(2):
                            for c in range(4):
                                nc.tensor.matmul(
                                    q_ps[:, blk, :], wq_t[:, c, blk * 128:(blk + 1) * 128],
                                    xt[:, c, :], start=(c == 0), stop=(c == 3))
                        # k+v matmuls: out [128 tok, 512 (kR kI v)] per t-tile
                        kv_ps = kvps.tile([128, 4, 512], F32, name="kvp")
                        for t in range(4):
                            for c in range(4):
                                nc.tensor.matmul(
                                    kv_ps[:, t, :], xt[:, c, t * 128:(t + 1) * 128],
                                    wkv_t[:, c, :], start=(c == 0), stop=False)
                            nc.tensor.matmul(kv_ps[:, t, :], ones_t[:], bk_t[:],
                                             start=False, stop=True)
                        # kv accumulation, two spans back (features guaranteed
                        # done; no PE stall)
                        if s > 1:
                            emit_kv(s - 2)

                        # psum -> sbuf copies (scalar), q gets bias folded in
                        q_sb = p1.tile([128, 2, 512], BF16, name="qsb")
                        for blk in range(2):
                            nc.scalar.activation(q_sb[:, blk, :], q_ps[:, blk, :],
                                                 AF.Identity, bias=bq_t[:, blk:blk + 1])
                        k_sb = p1.tile([128, 4, 256], BF16, name="ksb")
                        nc.scalar.copy(k_sb[:], kv_ps[:, :, 0:256])
                        va = kfp.tile([128, 4, 257], BF16, name="va")
                        nc.vector.memset(va[:, :, 256:257], 1.0)
                        nc.scalar.copy(va[:, :, 0:256], kv_ps[:, :, 256:512])

                        # ---- q rope (qT layout), paired ops ----
                        # tq1 = [c ; -s], tq2 = [s ; c]
                        P1 = p1.tile([128, 2, 512], BF16, name="P1")
                        P2 = p1.tile([128, 2, 512], BF16, name="P2")
                        nc.vector.tensor_tensor(P1[:], q_sb[:], tq1[:], op=ALU.mult)
                        nc.vector.tensor_tensor(P2[:], q_sb[:], tq2[:], op=ALU.mult)
                        rot_q = p1.tile([128, 2, 512], BF16, name="rotq")
                        nc.vector.tensor_tensor(rot_q[:, 0, :], P1[:, 0, :], P1[:, 1, :], op=ALU.add)
                        nc.vector.tensor_tensor(rot_q[:, 1, :], P2[:, 0, :], P2[:, 1, :], op=ALU.add)

                        # ---- k rope (token layout) ----
                        kR = k_sb[:, :, 0:128]
                        kI = k_sb[:, :, 128:256]
                        tk = p1.tile([128, 4, 512], BF16, name="tk")
                        tkv = [tk[:, i, :].rearrange("p (t d) -> p t d", d=128)
                               for i in range(4)]
                        nc.vector.tensor_tensor(tkv[0], kR, tkc[:], op=ALU.mult)
                        nc.vector.tensor_tensor(tkv[1], kI, tks[:], op=ALU.mult)
                        nc.vector.tensor_tensor(tkv[2], kR, tks[:], op=ALU.mult)
                        nc.vector.tensor_tensor(tkv[3], kI, tkc[:], op=ALU.mult)
                        rot_k = p1.tile([128, 2, 512], BF16, name="rotk")
                        nc.vector.tensor_tensor(rot_k[:, 0, :], tk[:, 0, :], tk[:, 1, :], op=ALU.subtract)
                        nc.vector.tensor_tensor(rot_k[:, 1, :], tk[:, 2, :], tk[:, 3, :], op=ALU.add)

                        rot_hist[s % 2] = (rot_q, rot_k)
                        va_hist[s % 3] = va
                        stage_feat(s)
                    emit_kv(nspan - 2)
                    emit_kv(nspan - 1)

                # ---- extract block-diag kv lhsT + den tiles via masks ----
                nc.vector.tensor_tensor(lR[0][:], kvR[:, 0:128], mask0_t[:], op=ALU.mult)
                nc.vector.tensor_tensor(lR[1][:], kvR[:, 128:256], mask1_t[:], op=ALU.mult)
                nc.vector.tensor_tensor(lI[0][:], kvI[:, 0:128], mask0_t[:], op=ALU.mult)
                nc.vector.tensor_tensor(lI[1][:], kvI[:, 128:256], mask1_t[:], op=ALU.mult)
                nc.vector.tensor_tensor(
                    denR[:], kvR[:, 256:257].broadcast_to([128, 4]), dmask_t[:], op=ALU.mult)
                nc.vector.tensor_tensor(
                    denI[:], kvI[:, 256:257].broadcast_to([128, 4]), dmask_t[:], op=ALU.mult)

            # ===== pass 2a: all denominators upfront (PE stays warm) =====
            # chunked tiles so 2b can start before all of 2a has finished
            zchunk = 8
            zinv_tiles = [store.tile([4, zchunk, 512], BF16, name=f"zinva{i}")
                          for i in range(nspan // zchunk)]
            with tc.tile_pool(name="p2a", bufs=3) as p2a, \
                 tc.tile_pool(name="dps", bufs=3, space="PSUM") as dps:
                for s in range(nspan):
                    sl = slice(s * 512, (s + 1) * 512)
                    den_ps = dps.tile([4, 512], F32, name="denp")
                    nc.tensor.matmul(den_ps[:], denR[:], q_store[:, 0, sl], start=True, stop=False)
                    nc.tensor.matmul(den_ps[:], denI[:], q_store[:, 1, sl], start=False, stop=True)
                    # 1/z = exp(-ln(z)); z is ~1e4 so edge cases are impossible
                    lnz = p2a.tile([4, 512], F32, name="lnz")
                    nc.scalar.activation(lnz[:], den_ps[:], AF.Ln)
                    nc.scalar.activation(zinv_tiles[s // zchunk][:, s % zchunk, :],
                                         lnz[:], AF.Exp, scale=-1.0)

            # ===== pass 2b: out, zb, proj (software pipelined) =====
            with tc.tile_pool(name="p2", bufs=3) as p2, \
                 tc.tile_pool(name="ops", bufs=2, space="PSUM") as ops, \
                 tc.tile_pool(name="zps", bufs=1, space="PSUM") as zps, \
                 tc.tile_pool(name="yps", bufs=2, space="PSUM") as yps:
                st = {}  # per-span live tiles

                def stage_a(s):
                    sl = slice(s * 512, (s + 1) * 512)
                    out_ps = ops.tile([128, 2, 512], F32, name="outp")
                    for i in range(2):
                        nc.tensor.matmul(out_ps[:, i, :], lR[i][:], q_store[:, 0, sl],
                                         start=True, stop=False)
                        nc.tensor.matmul(out_ps[:, i, :], lI[i][:], q_store[:, 1, sl],
                                         start=False, stop=True)
                    zb_ps = zps.tile([128, 2, 512], F32, name="zbp")
                    for i in range(2):
                        nc.tensor.matmul(zb_ps[:, i, :], sel_t[:, i, :],
                                         zinv_tiles[s // zchunk][:, s % zchunk, :],
                                         start=True, stop=True)
                    zb_sb = p2.tile([128, 2, 512], BF16, name="zbs")
                    nc.scalar.copy(zb_sb[:, 0, :], zb_ps[:, 0, :])
                    nc.vector.tensor_copy(zb_sb[:, 1, :], zb_ps[:, 1, :])
                    outT = p2.tile([128, 2, 512], BF16, name="outT")
                    for i in range(2):
                        nc.vector.tensor_tensor(outT[:, i, :], out_ps[:, i, :],
                                                zb_sb[:, i, :], op=ALU.mult)
                    st[s] = {"outT": outT}

                def stage_c(s):
                    d = st.pop(s)
                    outT = d["outT"]
                    y_sb = p2.tile([128, 4, 512], BF16, name="ysb")
                    for t in range(4):
                        y_ps = yps.tile([128, 512], F32, name="yp")
                        nc.tensor.matmul(y_ps[:], outT[:, 0, t * 128:(t + 1) * 128],
                                         wp_t[:, 0, :], start=True, stop=False)
                        nc.tensor.matmul(y_ps[:], outT[:, 1, t * 128:(t + 1) * 128],
                                         wp_t[:, 1, :], start=False, stop=True)
                        nc.scalar.copy(y_sb[:, t, :], y_ps[:])
                    nc.sync.dma_start(
                        y[s * 512:(s + 1) * 512, :].rearrange("(t p) c -> p t c", p=128),
                        y_sb[:])

                for s in range(nspan):
                    if s >= 2:
                        stage_c(s - 2)
                    stage_a(s)
                stage_c(nspan - 2)
                stage_c(nspan - 1)

    return nc


_NC_CACHE = {}


def _get_nc(n_tok):
    if n_tok not in _NC_CACHE:
        _NC_CACHE[n_tok] = _build_nc(n_tok)
    return _NC_CACHE[n_tok]


def _bf(a):
    return np.ascontiguousarray(np.asarray(a, dtype=np.float32)).astype(ml_dtypes.bfloat16)


_TABLES_CACHE = {}


def _tables(nspan):
    """Per-span rope tables, shared by all cores.

    qt1/qt2 [128 d, nspan, 2 blk, 512 tok']: q rotation in qT layout,
      qt1 = [cos ; -sin], qt2 = [sin ; cos].
    ktc/kts [128 tok, nspan, 4 t, 128 d]: k rotation in token layout.
    """
    if nspan in _TABLES_CACHE:
        return _TABLES_CACHE[nspan]
    j = np.arange(16)
    freqs = (1.0 / (THETA ** (4.0 * j / HD))).astype(np.float64)
    fcol = np.tile(freqs, 4)                      # [64] head-major 4h x 16f
    tx = np.arange(128).astype(np.float64)
    angx = np.outer(fcol, tx)                     # [64 f, 128 tx]
    rows = np.arange(nspan * 4).astype(np.float64)
    angy = np.outer(fcol, rows)                   # [64 f, nspan*4]

    # q tables [128, nspan, 4, 128]
    qc = np.empty((128, nspan, 4, 128), np.float32)
    qs_ = np.empty((128, nspan, 4, 128), np.float32)
    qc[0:64] = np.cos(angx)[:, None, None, :]
    qs_[0:64] = np.sin(angx)[:, None, None, :]
    qc[64:128] = np.cos(angy).reshape(64, nspan, 4, 1)
    qs_[64:128] = np.sin(angy).reshape(64, nspan, 4, 1)
    # stack blocks: [128, nspan, 2, 4*128]
    qcf = qc.reshape(128, nspan, 1, 512)
    qsf = qs_.reshape(128, nspan, 1, 512)
    qt1 = np.concatenate([qcf, -qsf], axis=2).reshape(128, nspan * 1024)
    qt2 = np.concatenate([qsf, qcf], axis=2).reshape(128, nspan * 1024)

    # k tables [128 tok(part), nspan, 4 t, 128 d]
    kc = np.empty((128, nspan, 4, 128), np.float32)
    ks = np.empty((128, nspan, 4, 128), np.float32)
    kc[:, :, :, 0:64] = np.cos(angx).T[:, None, None, :]
    ks[:, :, :, 0:64] = np.sin(angx).T[:, None, None, :]
    kc[:, :, :, 64:128] = np.cos(angy).T.reshape(1, nspan, 4, 64)
    ks[:, :, :, 64:128] = np.sin(angy).T.reshape(1, nspan, 4, 64)
    ktc = kc.reshape(128, nspan * 512)
    kts = ks.reshape(128, nspan * 512)
    out = (_bf(qt1), _bf(qt2), _bf(ktc), _bf(kts))
    _TABLES_CACHE[nspan] = out
    return out


def kernel(x, w_qkv, b_qkv, w_proj, b_proj, height, width):
    x = np.asarray(x); w_qkv = np.asarray(w_qkv); b_qkv = np.asarray(b_qkv)
    w_proj = np.asarray(w_proj); b_proj = np.asarray(b_proj)
    b, n, c = x.shape
    nc = _get_nc(n)
    qt1, qt2, ktc, kts = _tables(n // 512)

    sel = np.zeros((4, 2, 128), np.float32)
    for i in range(2):
        for h in range(2):
            sel[2 * i + h, i, 64 * h:64 * h + 64] = 1.0
    # head of partition p in the d layout: (p % 64) // 16
    hop = (np.arange(128) % 64) // 16
    mask0 = (hop[:, None] == (np.arange(128) // 64)[None, :]).astype(np.float32)
    mask1 = (hop[:, None] == (2 + np.arange(128) // 64)[None, :]).astype(np.float32)
    dmask = (hop[:, None] == np.arange(4)[None, :]).astype(np.float32)

    in_maps = []
    for core in range(8):
        bi, hg = core // 2, core % 2
        heads = [hg * NH + j for j in range(NH)]
        q0 = [h * HD + 2 * j for h in heads for j in range(16)] + \
             [h * HD + 32 + 2 * j for h in heads for j in range(16)]
        q1 = [cc + 1 for cc in q0]
        kR = [512 + cc for cc in q0]
        kI = [512 + cc for cc in q1]
        vc = [1024 + h * HD + e for h in heads for e in range(HD)]
        wq_cols = q0 + q1
        wkv_cols = kR + kI + vc
        bk = np.concatenate([b_qkv[kR + kI], np.zeros(256, np.float32)])
        in_maps.append({
            "xT": _bf(x[bi].T.reshape(4, 128, n // 512, 512).transpose(1, 2, 0, 3).reshape(128, -1)),
            "w_q": _bf(w_qkv[:, wq_cols]).reshape(4, 128, 256),
            "w_kv": _bf(w_qkv[:, wkv_cols]).reshape(4, 128, 512),
            "bq": np.stack([b_qkv[q0], b_qkv[q1]], axis=1).astype(np.float32),
            "bkrow": _bf(bk)[None, :],
            "wp": _bf(np.stack([w_proj[hg * 256:hg * 256 + 128, :],
                                w_proj[hg * 256 + 128:hg * 256 + 256, :]])),
            "qt1": qt1, "qt2": qt2, "ktc": ktc, "kts": kts,
            "sel": _bf(sel),
            "mask0": _bf(mask0), "mask1": _bf(mask1), "dmask": _bf(dmask),
        })
    res = run_bass_kernel_spmd(nc, in_maps, list(range(8)), trace=False)
    bias_eff = (b_proj.astype(np.float32)
                + b_qkv[1024:].astype(np.float32) @ w_proj.astype(np.float32))
    out = np.empty((b, n, c), np.float32)
    for bi in range(b):
        out[bi] = (res.results[2 * bi]["y"].astype(np.float32)
                   + res.results[2 * bi + 1]["y"].astype(np.float32)
                   + bias_eff[None, :])
    return out


# revision 43
# speedup vs baseline: 1.2682x; 1.0619x over previous
import sys

sys.path.insert(0, "/opt/trn_rl_repo")
import numpy as np
import ml_dtypes
import concourse.bass as bass
import concourse.mybir as mybir
import concourse.tile as tile
from concourse.bass_utils import run_bass_kernel_spmd

F32 = mybir.dt.float32
BF16 = mybir.dt.bfloat16
AF = mybir.ActivationFunctionType
ALU = mybir.AluOpType

C = 512
NH = 4          # heads per core (8 global, split in 2 groups of 4)
HD = 64
THETA = 10.0


import json as _json
import concourse.bass2jax as _b2j
import concourse.bass_utils as _bu

_ORIG_COMPILE = _bu.compile_bir_kernel


def _patched_compile_bir_kernel(bir_json, tmpdir, neff_name="file.neff"):
    """This walrus rejects instructions whose sync waits+updates exceed 2.
    Rewrite the BIR: move excess waits onto inserted same-engine Drains."""
    d = _json.loads(bir_json)
    for fn in d.get("functions", []):
        for b in fn.get("blocks", []):
            out = []
            for i in b.get("instructions", []):
                si = i.get("sync_info")
                if si:
                    ow = si.get("on_wait") or []
                    ou = si.get("on_update") or []
                    cap = 1 if i.get("opcode") in ("Drain", "Ldweights") else 2
                    budget = cap - len(ou)
                    if len(ow) > budget:
                        keep = ow[-budget:] if budget > 0 else []
                        extra = ow[:-budget] if budget > 0 else ow
                        for ci, w in enumerate(extra):
                            out.append({
                                "debug": i.get("debug", 0),
                                "engine": i["engine"],
                                "ins": [], "outs": [],
                                "name": f"{i['name']}sw{ci}",
                                "opcode": "Drain",
                                "sync_info": {"on_update": [],
                                              "on_wait": [w]},
                            })
                        si["on_wait"] = keep
                out.append(i)
            b["instructions"] = out
    return _ORIG_COMPILE(_json.dumps(d).encode(), tmpdir, neff_name=neff_name)


_bu.compile_bir_kernel = _patched_compile_bir_kernel
_b2j.compile_bir_kernel = _patched_compile_bir_kernel


def _build_nc(n_tok):
    nspan = n_tok // 512
    nc = bass.Bass()
    xT = nc.declare_dram_parameter("xT", [128, (n_tok // 512) * 2048], BF16, isOutput=False)
    w_q = nc.declare_dram_parameter("w_q", [4, 128, 256], BF16, isOutput=False)
    w_kv = nc.declare_dram_parameter("w_kv", [4, 128, 512], BF16, isOutput=False)
    bq = nc.declare_dram_parameter("bq", [128, 2], F32, isOutput=False)
    bkrow = nc.declare_dram_parameter("bkrow", [1, 512], BF16, isOutput=False)
    wp = nc.declare_dram_parameter("wp", [2, 128, 512], BF16, isOutput=False)
    # per-span rope tables (precomputed on host, DMA'd per span)
    qt1 = nc.declare_dram_parameter("qt1", [128, nspan * 1024], BF16, isOutput=False)
    qt2 = nc.declare_dram_parameter("qt2", [128, nspan * 1024], BF16, isOutput=False)
    ktc = nc.declare_dram_parameter("ktc", [128, nspan * 512], BF16, isOutput=False)
    kts = nc.declare_dram_parameter("kts", [128, nspan * 512], BF16, isOutput=False)
    sel = nc.declare_dram_parameter("sel", [4, 2, 128], BF16, isOutput=False)
    mask0 = nc.declare_dram_parameter("mask0", [128, 128], BF16, isOutput=False)
    mask1 = nc.declare_dram_parameter("mask1", [128, 128], BF16, isOutput=False)
    dmask = nc.declare_dram_parameter("dmask", [128, 4], BF16, isOutput=False)
    y = nc.declare_dram_parameter("y", [n_tok, 512], BF16, isOutput=True)

    with nc.allow_low_precision(reason="bf16 pipeline by design"), tile.TileContext(nc) as tc:
        with tc.tile_pool(name="wpool", bufs=1) as wpool, \
             tc.tile_pool(name="store", bufs=1) as store:
            # ---- persistent tiles ----
            wq_t = wpool.tile([128, 4, 256], BF16, name="wq")
            wkv_t = wpool.tile([128, 4, 512], BF16, name="wkv")
            bq_t = wpool.tile([128, 2], F32, name="bq")
            bk_t = wpool.tile([1, 512], BF16, name="bk")
            wp_t = wpool.tile([128, 2, 512], BF16, name="wp")
            sel_t = wpool.tile([4, 2, 128], BF16, name="sel")
            mask0_t = wpool.tile([128, 128], BF16, name="mask0")
            mask1_t = wpool.tile([128, 128], BF16, name="mask1")
            dmask_t = wpool.tile([128, 4], BF16, name="dmask")
            ones_t = wpool.tile([1, 128], BF16, name="ones")

            nc.sync.dma_start(wq_t[:], w_q.rearrange("c p d -> p c d"))
            nc.sync.dma_start(wkv_t[:], w_kv.rearrange("c p d -> p c d"))
            nc.sync.dma_start(bq_t[:], bq[:])
            nc.sync.dma_start(bk_t[:], bkrow[:])
            nc.sync.dma_start(wp_t[:], wp.rearrange("e p c -> p e c"))
            nc.sync.dma_start(sel_t[:], sel[:])
            nc.sync.dma_start(mask0_t[:], mask0[:])
            nc.sync.dma_start(mask1_t[:], mask1[:])
            nc.sync.dma_start(dmask_t[:], dmask[:])
            nc.vector.memset(ones_t[:], 1.0)

            # q features for the whole sequence: [128, 2(R/I), n_tok] bf16
            q_store = store.tile([128, 2, n_tok], BF16, name="qs")

            # kv lhsT + den tiles (filled after pass 1)
            lR = [wpool.tile([128, 128], BF16, name=f"lR{i}") for i in range(2)]
            lI = [wpool.tile([128, 128], BF16, name=f"lI{i}") for i in range(2)]
            denR = wpool.tile([128, 4], BF16, name="denR")
            denI = wpool.tile([128, 4], BF16, name="denI")

            # ================ pass 1 ================
            with tc.tile_pool(name="kvacc", bufs=1, space="PSUM") as kvacc:
                kvR = kvacc.tile([128, 257], F32, name="kvR")
                kvI = kvacc.tile([128, 257], F32, name="kvI")
                with tc.tile_pool(name="p1", bufs=2) as p1, \
                     tc.tile_pool(name="kfp", bufs=3) as kfp, \
                     tc.tile_pool(name="xp", bufs=3) as xp, \
                     tc.tile_pool(name="qps", bufs=1, space="PSUM") as qps, \
                     tc.tile_pool(name="kvps", bufs=1, space="PSUM") as kvps:
                    kf_hist = [None, None, None]
                    va_hist = [None, None, None]
                    rot_hist = [None, None]
                    xt_tiles = {}
                    tab_tiles = {}

                    def dma_xt(s):
                        xt = xp.tile([128, 4, 512], BF16, name="xt")
                        nc.sync.dma_start(
                            xt[:], xT[:, s * 2048:(s + 1) * 2048].rearrange("p (c t) -> p c t", c=4))
                        xt_tiles[s] = xt

                    def dma_tab(s):
                        tq1 = p1.tile([128, 2, 512], BF16, name="tq1")
                        tq2 = p1.tile([128, 2, 512], BF16, name="tq2")
                        tkc = p1.tile([128, 4, 128], BF16, name="tkc")
                        tks = p1.tile([128, 4, 128], BF16, name="tks")
                        nc.sync.dma_start(
                            tq1[:], qt1[:, s * 1024:(s + 1) * 1024].rearrange("p (b t) -> p b t", b=2))
                        nc.sync.dma_start(
                            tq2[:], qt2[:, s * 1024:(s + 1) * 1024].rearrange("p (b t) -> p b t", b=2))
                        nc.sync.dma_start(
                            tkc[:], ktc[:, s * 512:(s + 1) * 512].rearrange("p (t d) -> p t d", t=4))
                        nc.sync.dma_start(
                            tks[:], kts[:, s * 512:(s + 1) * 512].rearrange("p (t d) -> p t d", t=4))
                        tab_tiles[s] = (tq1, tq2, tkc, tks)

                    def stage_feat(s):
                        # elu(x)+1 = min(exp(x), max(x+1, 1))
                        rot_q, rot_k = rot_hist[s % 2]
                        exp_q = p1.tile([128, 2, 512], BF16, name="expq")
                        exp_k = p1.tile([128, 2, 512], BF16, name="expk")
                        nc.scalar.activation(exp_q[:], rot_q[:], AF.Exp)
                        nc.scalar.activation(exp_k[:], rot_k[:], AF.Exp)
                        a_q = p1.tile([128, 2, 512], BF16, name="aq")
                        a_k = p1.tile([128, 2, 512], BF16, name="ak")
                        nc.vector.tensor_scalar(a_q[:], rot_q[:], 1.0, 1.0,
                                                op0=ALU.add, op1=ALU.max)
                        nc.vector.tensor_scalar(a_k[:], rot_k[:], 1.0, 1.0,
                                                op0=ALU.add, op1=ALU.max)
                        nc.vector.tensor_tensor(
                            q_store[:, :, s * 512:(s + 1) * 512], exp_q[:], a_q[:], op=ALU.min)
                        # kf layout: [128 tok, 2 (R/I), 512 (t*128+d)]
                        kf = kfp.tile([128, 2, 512], BF16, name="kf")
                        nc.vector.tensor_tensor(kf[:], exp_k[:], a_k[:], op=ALU.min)
                        kf_hist[s % 3] = kf

                    def emit_kv(s):
                        bs = s % 3
                        kf = kf_hist[bs]
                        va = va_hist[bs]
                        for t in range(4):
                            st = (s == 0 and t == 0)
                            sp = (s == nspan - 1 and t == 3)
                            nc.tensor.matmul(kvR[:], kf[:, 0, t * 128:(t + 1) * 128],
                                             va[:, t, :], start=st, stop=sp)
                            nc.tensor.matmul(kvI[:], kf[:, 1, t * 128:(t + 1) * 128],
                                             va[:, t, :], start=st, stop=sp)

                    # prefetch: x tiles 2 spans ahead, tables 1 span ahead
                    dma_xt(0)
                    dma_xt(1)
                    dma_tab(0)
                    for s in range(nspan):
                        if s + 2 < nspan:
                            dma_xt(s + 2)
                        if s + 1 < nspan:
                            dma_tab(s + 1)
                        xt = xt_tiles.pop(s)
                        tq1, tq2, tkc, tks = tab_tiles.pop(s)

                        # q matmuls: out [128 d, 512 tok] per block, lhsT = w chunks
                        q_ps = qps.tile([128, 2, 512], F32, name="qp")
                        for blk in range(2):
                            for c in range(4):
                                nc.tensor.matmul(
                                    q_ps[:, blk, :], wq_t[:, c, blk * 128:(blk + 1) * 128],
                                    xt[:, c, :], start=(c == 0), stop=(c == 3))
                        # k+v matmuls: out [128 tok, 512 (kR kI v)] per t-tile
                        kv_ps = kvps.tile([128, 4, 512], F32, name="kvp")
                        for t in range(4):
                            for c in range(4):
                                nc.tensor.matmul(
                                    kv_ps[:, t, :], xt[:, c, t * 128:(t + 1) * 128],
                                    wkv_t[:, c, :], start=(c == 0), stop=False)
                            nc.tensor.matmul(kv_ps[:, t, :], ones_t[:], bk_t[:],
                                             start=False, stop=True)
                        # kv accumulation, two spans back (features guaranteed
                        # done; no PE stall)
                        if s > 1:
                            emit_kv(s - 2)

                        # psum -> sbuf copies (scalar), q gets bias folded in
                        q_sb = p1.tile([128, 2, 512], BF16, name="qsb")
                        for blk in range(2):
                            nc.scalar.activation(q_sb[:, blk, :], q_ps[:, blk, :],
                                                 AF.Identity, bias=bq_t[:, blk:blk + 1])
                        k_sb = p1.tile([128, 4, 256], BF16, name="ksb")
                        nc.scalar.copy(k_sb[:], kv_ps[:, :, 0:256])
                        va = kfp.tile([128, 4, 257], BF16, name="va")
                        nc.vector.memset(va[:, :, 256:257], 1.0)
                        nc.scalar.copy(va[:, :, 0:256], kv_ps[:, :, 256:512])

                        # ---- q rope (qT layout), paired ops ----
                        # tq1 = [c ; -s], tq2 = [s ; c]
                        P1 = p1.tile([128, 2, 512], BF16, name="P1")
                        P2 = p1.tile([128, 2, 512], BF16, name="P2")
                        nc.vector.tensor_tensor(P1[:], q_sb[:], tq1[:], op=ALU.mult)
                        nc.vector.tensor_tensor(P2[:], q_sb[:], tq2[:], op=ALU.mult)
                        rot_q = p1.tile([128, 2, 512], BF16, name="rotq")
                        nc.vector.tensor_tensor(rot_q[:, 0, :], P1[:, 0, :], P1[:, 1, :], op=ALU.add)
                        nc.vector.tensor_tensor(rot_q[:, 1, :], P2[:, 0, :], P2[:, 1, :], op=ALU.add)

                        # ---- k rope (token layout) ----
                        kR = k_sb[:, :, 0:128]
                        kI = k_sb[:, :, 128:256]
                        tk = p1.tile([128, 4, 512], BF16, name="tk")
                        tkv = [tk[:, i, :].rearrange("p (t d) -> p t d", d=128)
                               for i in range(4)]
                        nc.vector.tensor_tensor(tkv[0], kR, tkc[:], op=ALU.mult)
                        nc.vector.tensor_tensor(tkv[1], kI, tks[:], op=ALU.mult)
                        nc.vector.tensor_tensor(tkv[2], kR, tks[:], op=ALU.mult)
                        nc.vector.tensor_tensor(tkv[3], kI, tkc[:], op=ALU.mult)
                        rot_k = p1.tile([128, 2, 512], BF16, name="rotk")
                        nc.vector.tensor_tensor(rot_k[:, 0, :], tk[:, 0, :], tk[:, 1, :], op=ALU.subtract)
                        nc.vector.tensor_tensor(rot_k[:, 1, :], tk[:, 2, :], tk[:, 3, :], op=ALU.add)

                        rot_hist[s % 2] = (rot_q, rot_k)
                        va_hist[s % 3] = va
                        stage_feat(s)
                    emit_kv(nspan - 2)
                    emit_kv(nspan - 1)

                # ---- extract block-diag kv lhsT + den tiles via masks ----
                nc.vector.tensor_tensor(lR[0][:], kvR[:, 0:128], mask0_t[:], op=ALU.mult)
                nc.vector.tensor_tensor(lR[1][:], kvR[:, 128:256], mask1_t[:], op=ALU.mult)
                nc.vector.tensor_tensor(lI[0][:], kvI[:, 0:128], mask0_t[:], op=ALU.mult)
                nc.vector.tensor_tensor(lI[1][:], kvI[:, 128:256], mask1_t[:], op=ALU.mult)
                nc.vector.tensor_tensor(
                    denR[:], kvR[:, 256:257].broadcast_to([128, 4]), dmask_t[:], op=ALU.mult)
                nc.vector.tensor_tensor(
                    denI[:], kvI[:, 256:257].broadcast_to([128, 4]), dmask_t[:], op=ALU.mult)

            # ===== pass 2: den interleaved 4 spans ahead of out/zb/proj =====
            zchunk = 8
            zinv_tiles = [store.tile([4, zchunk, 512], BF16, name=f"zinva{i}")
                          for i in range(nspan // zchunk)]
            with tc.tile_pool(name="p2", bufs=3) as p2, \
                 tc.tile_pool(name="dps", bufs=2, space="PSUM") as dps, \
                 tc.tile_pool(name="ops", bufs=1, space="PSUM") as ops, \
                 tc.tile_pool(name="zps", bufs=1, space="PSUM") as zps, \
                 tc.tile_pool(name="yps", bufs=2, space="PSUM") as yps:
                st = {}  # per-span live tiles

                def stage_den(s):
                    sl = slice(s * 512, (s + 1) * 512)
                    den_ps = dps.tile([4, 512], F32, name="denp")
                    nc.tensor.matmul(den_ps[:], denR[:], q_store[:, 0, sl], start=True, stop=False)
                    nc.tensor.matmul(den_ps[:], denI[:], q_store[:, 1, sl], start=False, stop=True)
                    # 1/z = exp(-ln(z)); z is ~1e4 so edge cases are impossible
                    lnz = p2.tile([4, 512], F32, name="lnz")
                    nc.scalar.activation(lnz[:], den_ps[:], AF.Ln)
                    nc.scalar.activation(zinv_tiles[s // zchunk][:, s % zchunk, :],
                                         lnz[:], AF.Exp, scale=-1.0)

                def stage_a(s):
                    sl = slice(s * 512, (s + 1) * 512)
                    out_ps = ops.tile([128, 2, 512], F32, name="outp")
                    for i in range(2):
                        nc.tensor.matmul(out_ps[:, i, :], lR[i][:], q_store[:, 0, sl],
                                         start=True, stop=False)
                        nc.tensor.matmul(out_ps[:, i, :], lI[i][:], q_store[:, 1, sl],
                                         start=False, stop=True)
                    zb_ps = zps.tile([128, 2, 512], F32, name="zbp")
                    for i in range(2):
                        nc.tensor.matmul(zb_ps[:, i, :], sel_t[:, i, :],
                                         zinv_tiles[s // zchunk][:, s % zchunk, :],
                                         start=True, stop=True)
                    zb_sb = p2.tile([128, 2, 512], BF16, name="zbs")
                    nc.scalar.copy(zb_sb[:, 0, :], zb_ps[:, 0, :])
                    nc.vector.tensor_copy(zb_sb[:, 1, :], zb_ps[:, 1, :])
                    outT = p2.tile([128, 2, 512], BF16, name="outT")
                    for i in range(2):
                        nc.vector.tensor_tensor(outT[:, i, :], out_ps[:, i, :],
                                                zb_sb[:, i, :], op=ALU.mult)
                    st[s] = {"outT": outT}

                def stage_c(s):
                    d = st.pop(s)
                    outT = d["outT"]
                    y_sb = p2.tile([128, 4, 512], BF16, name="ysb")
                    for t in range(4):
                        y_ps = yps.tile([128, 512], F32, name="yp")
                        nc.tensor.matmul(y_ps[:], outT[:, 0, t * 128:(t + 1) * 128],
                                         wp_t[:, 0, :], start=True, stop=False)
                        nc.tensor.matmul(y_ps[:], outT[:, 1, t * 128:(t + 1) * 128],
                                         wp_t[:, 1, :], start=False, stop=True)
                        nc.scalar.copy(y_sb[:, t, :], y_ps[:])
                    nc.sync.dma_start(
                        y[s * 512:(s + 1) * 512, :].rearrange("(t p) c -> p t c", p=128),
                        y_sb[:])

                for s in range(4):
                    stage_den(s)
                for s in range(nspan):
                    if s + 4 < nspan:
                        stage_den(s + 4)
                    if s >= 2:
                        stage_c(s - 2)
                    stage_a(s)
                stage_c(nspan - 2)
                stage_c(nspan - 1)

    return nc


_NC_CACHE = {}


def _get_nc(n_tok):
    if n_tok not in _NC_CACHE:
        _NC_CACHE[n_tok] = _build_nc(n_tok)
    return _NC_CACHE[n_tok]


def _bf(a):
    return np.ascontiguousarray(np.asarray(a, dtype=np.float32)).astype(ml_dtypes.bfloat16)


_TABLES_CACHE = {}


def _tables(nspan):
    """Per-span rope tables, shared by all cores.

    qt1/qt2 [128 d, nspan, 2 blk, 512 tok']: q rotation in qT layout,
      qt1 = [cos ; -sin], qt2 = [sin ; cos].
    ktc/kts [128 tok, nspan, 4 t, 128 d]: k rotation in token layout.
    """
    if nspan in _TABLES_CACHE:
        return _TABLES_CACHE[nspan]
    j = np.arange(16)
    freqs = (1.0 / (THETA ** (4.0 * j / HD))).astype(np.float64)
    fcol = np.tile(freqs, 4)                      # [64] head-major 4h x 16f
    tx = np.arange(128).astype(np.float64)
    angx = np.outer(fcol, tx)                     # [64 f, 128 tx]
    rows = np.arange(nspan * 4).astype(np.float64)
    angy = np.outer(fcol, rows)                   # [64 f, nspan*4]

    # q tables [128, nspan, 4, 128]
    qc = np.empty((128, nspan, 4, 128), np.float32)
    qs_ = np.empty((128, nspan, 4, 128), np.float32)
    qc[0:64] = np.cos(angx)[:, None, None, :]
    qs_[0:64] = np.sin(angx)[:, None, None, :]
    qc[64:128] = np.cos(angy).reshape(64, nspan, 4, 1)
    qs_[64:128] = np.sin(angy).reshape(64, nspan, 4, 1)
    # stack blocks: [128, nspan, 2, 4*128]
    qcf = qc.reshape(128, nspan, 1, 512)
    qsf = qs_.reshape(128, nspan, 1, 512)
    qt1 = np.concatenate([qcf, -qsf], axis=2).reshape(128, nspan * 1024)
    qt2 = np.concatenate([qsf, qcf], axis=2).reshape(128, nspan * 1024)

    # k tables [128 tok(part), nspan, 4 t, 128 d]
    kc = np.empty((128, nspan, 4, 128), np.float32)
    ks = np.empty((128, nspan, 4, 128), np.float32)
    kc[:, :, :, 0:64] = np.cos(angx).T[:, None, None, :]
    ks[:, :, :, 0:64] = np.sin(angx).T[:, None, None, :]
    kc[:, :, :, 64:128] = np.cos(angy).T.reshape(1, nspan, 4, 64)
    ks[:, :, :, 64:128] = np.sin(angy).T.reshape(1, nspan, 4, 64)
    ktc = kc.reshape(128, nspan * 512)
    kts = ks.reshape(128, nspan * 512)
    out = (_bf(qt1), _bf(qt2), _bf(ktc), _bf(kts))
    _TABLES_CACHE[nspan] = out
    return out


def kernel(x, w_qkv, b_qkv, w_proj, b_proj, height, width):
    x = np.asarray(x); w_qkv = np.asarray(w_qkv); b_qkv = np.asarray(b_qkv)
    w_proj = np.asarray(w_proj); b_proj = np.asarray(b_proj)
    b, n, c = x.shape
    nc = _get_nc(n)
    qt1, qt2, ktc, kts = _tables(n // 512)

    sel = np.zeros((4, 2, 128), np.float32)
    for i in range(2):
        for h in range(2):
            sel[2 * i + h, i, 64 * h:64 * h + 64] = 1.0
    # head of partition p in the d layout: (p % 64) // 16
    hop = (np.arange(128) % 64) // 16
    mask0 = (hop[:, None] == (np.arange(128) // 64)[None, :]).astype(np.float32)
    mask1 = (hop[:, None] == (2 + np.arange(128) // 64)[None, :]).astype(np.float32)
    dmask = (hop[:, None] == np.arange(4)[None, :]).astype(np.float32)

    in_maps = []
    for core in range(8):
        bi, hg = core // 2, core % 2
        heads = [hg * NH + j for j in range(NH)]
        q0 = [h * HD + 2 * j for h in heads for j in range(16)] + \
             [h * HD + 32 + 2 * j for h in heads for j in range(16)]
        q1 = [cc + 1 for cc in q0]
        kR = [512 + cc for cc in q0]
        kI = [512 + cc for cc in q1]
        vc = [1024 + h * HD + e for h in heads for e in range(HD)]
        wq_cols = q0 + q1
        wkv_cols = kR + kI + vc
        bk = np.concatenate([b_qkv[kR + kI], np.zeros(256, np.float32)])
        in_maps.append({
            "xT": _bf(x[bi].T.reshape(4, 128, n // 512, 512).transpose(1, 2, 0, 3).reshape(128, -1)),
            "w_q": _bf(w_qkv[:, wq_cols]).reshape(4, 128, 256),
            "w_kv": _bf(w_qkv[:, wkv_cols]).reshape(4, 128, 512),
            "bq": np.stack([b_qkv[q0], b_qkv[q1]], axis=1).astype(np.float32),
            "bkrow": _bf(bk)[None, :],
            "wp": _bf(np.stack([w_proj[hg * 256:hg * 256 + 128, :],
                                w_proj[hg * 256 + 128:hg * 256 + 256, :]])),
            "qt1": qt1, "qt2": qt2, "ktc": ktc, "kts": kts,
            "sel": _bf(sel),
            "mask0": _bf(mask0), "mask1": _bf(mask1), "dmask": _bf(dmask),
        })
    res = run_bass_kernel_spmd(nc, in_maps, list(range(8)), trace=False)
    bias_eff = (b_proj.astype(np.float32)
                + b_qkv[1024:].astype(np.float32) @ w_proj.astype(np.float32))
    out = np.empty((b, n, c), np.float32)
    for bi in range(b):
        out[bi] = (res.results[2 * bi]["y"].astype(np.float32)
                   + res.results[2 * bi + 1]["y"].astype(np.float32)
                   + bias_eff[None, :])
    return out
